# revision 23
# baseline (speedup 1.0000x reference)
"""BiLSTM-CRF Trainium2 kernel (Bass/Tile), three SPMD launches.

The 512-step LSTM recurrence and the 512-step CRF Viterbi scan are both
latency-chain bound on-chip (each step is a serial PE->ACT->DVE chain).
Both recurrences are exponentially forgetting, so they are chunked across
all 8 cores with warm-up prefixes that converge to the sequential
trajectory far below the (already path-exact) bf16 noise floor:

  L1 (8 cores): per direction, 32 LSTM chunks of 15 steps with a 32-step
      zero-state warm-up (chunk 0 starts from the true h0/c0 and is 47
      long). Each core runs its 8 chunks in lockstep: the chain index is
      a free-axis column, so one Ldweights+Matmult pair serves all 8
      chains ([128,8] moving operand) and the sigmoid/tanh/elementwise
      ops batch across chains. tanh(g) is computed as 2*sigmoid(2g)-1
      (g weights pre-scaled by 2 on the host, exact in bf16), so a step
      is: one sigmoid over i,f,g, one sigmoid over o, 4 DVE ops, one
      tanh, one h-multiply.
  L2 (8 cores): CRF Viterbi scan in 8 chunks of 62 steps with a 16-step
      zero-state warm-up (chunk 0 is 78 long, exact START init). Each
      core then composes its chunk's backpointer one-hot maps into
      suffix products on PE (6 concurrent sub-chains), emitting per-step
      "tag given chunk-end-tag" columns and the whole-chunk map G.
  L3 (1 core): chains the 8 chunk maps G to pick each chunk's end tag,
      then selects each chunk's tag columns with one matmul per chunk.

Host work is sharding glue: dtype casts, weight re-layout (incl. the x2
g-gate scaling), window slicing, time reversal for the backward
direction, and output concat.
"""

import numpy as np
from contextlib import ExitStack

import concourse.bass as bass
import concourse.tile as tile
from concourse import bacc, mybir
from concourse.bass_utils import run_bass_kernel_spmd
from concourse.masks import make_identity

F32 = mybir.dt.float32
I32 = mybir.dt.int32
U32 = mybir.dt.uint32
BF16 = mybir.dt.bfloat16
AF = mybir.ActivationFunctionType
OP = mybir.AluOpType

V, E, H, L = 100000, 300, 512, 512
NT, START, STOP, NEG = 20, 18, 19, -10000.0
G4 = 4 * H   # 2048
NM = G4 // 128  # 16 gate column-chunks
NK = H // 128   # 4 h row-chunks

# --- L1 chunking: 32 chunks per direction, 4 cores x 8 lockstep chains ---
NCH = 8       # chains per core (lockstep, chain = free-axis column)
W1 = 32       # LSTM warm-up steps
S1 = 47       # steps per chain (= W1 + CL1 = chunk-0 length)
CL1 = S1 - W1  # 15
SP1 = 48      # padded per-chain stride in the gathered window (8*48=384)
NG = 3        # gather tiles of 128 positions
NPOS = NG * 128  # 384
assert S1 + 31 * CL1 == L and NCH * SP1 == NPOS

# --- L2 chunking: 8 chunks, 1 per core ---
W2 = 16
S2 = 78       # scan steps per core (= W2 + CL2 = chunk-0 length)
CL2 = S2 - W2  # 62
SP2 = 80      # padded hcat window stride
NSUB = 6      # composition sub-chains
SUB = [(13 * u, 13 * (u + 1)) for u in range(NSUB)]
assert S2 + 7 * CL2 == L and SUB[-1][1] == S2

# gate row order used on-chip: g, i, f, o (sigmoid over g2 fires first)
_PERM = np.concatenate([
    np.arange(2 * H, 3 * H),  # g
    np.arange(0, H),          # i
    np.arange(H, 2 * H),      # f
    np.arange(3 * H, 4 * H),  # o
])

_CACHE: dict = {}


def _new_nc(num_devices):
    return bacc.Bacc(
        "TRN2", target_bir_lowering=False, debug=False, num_devices=num_devices
    )


# --------------------------------------------------------------------------
# L1: gather + input projection + 8 lockstep LSTM chunk recurrences
# --------------------------------------------------------------------------
def build_l1():
    nc = _new_nc(8)
    emb_d = nc.dram_tensor("emb", [V, E], F32, kind="ExternalInput").ap()
    sent_d = nc.dram_tensor("sent", [128, NG], I32, kind="ExternalInput").ap()
    wA_d = nc.dram_tensor("wA", [128, 2 * G4], BF16, kind="ExternalInput").ap()
    wB_d = nc.dram_tensor("wB", [E - 256, G4], BF16, kind="ExternalInput").ap()
    wC_d = nc.dram_tensor("wC", [1, G4], BF16, kind="ExternalInput").ap()
    wp_d = nc.dram_tensor("wpack", [128, NK * G4], BF16, kind="ExternalInput").ap()
    h0_d = nc.dram_tensor("h0c", [128, NK * NCH], BF16, kind="ExternalInput").ap()
    c0_d = nc.dram_tensor("c0c", [128, NK * NCH], F32, kind="ExternalInput").ap()
    hT_d = nc.dram_tensor("hT_out", [128, S1 * NK * NCH], BF16,
                          kind="ExternalOutput").ap()

    with tile.TileContext(nc) as tc, ExitStack() as ctx:
        const = ctx.enter_context(tc.tile_pool(name="const", bufs=1))
        state = ctx.enter_context(tc.tile_pool(name="state", bufs=1))
        ew = ctx.enter_context(tc.tile_pool(name="ew", bufs=2))

        ident = const.tile([128, 128], F32)
        make_identity(nc, ident[:])
        # x-projection, one tile per gate group so the recurrence can
        # start as soon as the g-group columns are written:
        # xpG (g: m 0..4), xpIF (i,f: m 4..12), xpB (o: m 12..16),
        # layout [128, (t*gm + (m-m0))*NCH + c]
        xpG = const.tile([128, SP1 * 4 * NCH], F32)
        xpIF = const.tile([128, SP1 * 8 * NCH], F32)
        xpB = const.tile([128, SP1 * 4 * NCH], F32)

        # --- phase A: gather + transpose + input projection ---
        # (index DMA first so the gathers don't queue behind the weights)
        phase_a = ExitStack()
        pxp = phase_a.enter_context(tc.tile_pool(name="pxp", bufs=2, space="PSUM"))
        ptp = phase_a.enter_context(tc.tile_pool(name="ptp", bufs=1, space="PSUM"))
        ones = const.tile([1, NPOS], BF16)
        nc.gpsimd.memset(ones[:], 1.0)
        idx = const.tile([128, NG], I32)
        nc.sync.dma_start(idx[:], sent_d[:, :])
        xg = []
        for g in range(NG):
            t = const.tile([128, E], F32, tag=f"xg{g}", name=f"xg{g}")
            nc.gpsimd.indirect_dma_start(
                out=t[:], out_offset=None, in_=emb_d[:, :],
                in_offset=bass.IndirectOffsetOnAxis(ap=idx[:, g:g + 1], axis=0),
            )
            xg.append(t)
        ecs = [128, 128, E - 256]
        xT = const.tile([128, 3 * NPOS], BF16)
        for e in range(3):
            e0 = sum(ecs[:e])
            for g in range(NG):
                pt = ptp.tile([128, 128], F32, space="PSUM", tag="pt")
                nc.tensor.transpose(
                    out=pt[0:ecs[e], :], in_=xg[g][:, e0:e0 + ecs[e]],
                    identity=ident[:],
                )
                nc.vector.tensor_copy(
                    xT[0:ecs[e], e * NPOS + g * 128: e * NPOS + (g + 1) * 128],
                    pt[0:ecs[e], :],
                )
        # weight loads go through the same (Pool/SWDGE) queue as the
        # gathers, AFTER them, so the gathers aren't stuck behind 3 MB of
        # weights on the DMA engines. rowi artificially depends on idx so
        # the scheduler cannot hoist the weight loads above the gathers.
        rowi0 = const.tile([128, 1], I32)
        nc.gpsimd.iota(rowi0[:], pattern=[[0, 1]], base=0, channel_multiplier=1)
        rowi = const.tile([128, 1], I32)
        nc.vector.scalar_tensor_tensor(
            out=rowi[:], in0=idx[:, 0:1], scalar=0, in1=rowi0[:],
            op0=OP.mult, op1=OP.add,
        )
        wa_sb = const.tile([128, 2 * G4], BF16)
        nc.gpsimd.indirect_dma_start(
            out=wa_sb[:], out_offset=None, in_=wA_d[:, :],
            in_offset=bass.IndirectOffsetOnAxis(ap=rowi[:, 0:1], axis=0))
        wb_sb = const.tile([E - 256, G4], BF16)
        nc.gpsimd.indirect_dma_start(
            out=wb_sb[:], out_offset=None, in_=wB_d[:, :],
            in_offset=bass.IndirectOffsetOnAxis(ap=rowi[0:E - 256, 0:1], axis=0))
        wc_sb = const.tile([1, G4], BF16)
        nc.sync.dma_start(wc_sb[:], wC_d[:, :])
        wp = const.tile([128, NK * G4], BF16)
        nc.gpsimd.indirect_dma_start(
            out=wp[:], out_offset=None, in_=wp_d[:, :],
            in_offset=bass.IndirectOffsetOnAxis(ap=rowi[:, 0:1], axis=0))
        xpvG = xpG[:].rearrange("p (t m c) -> p t m c", m=4, c=NCH)
        xpvIF = xpIF[:].rearrange("p (t m c) -> p t m c", m=8, c=NCH)
        xpvB = xpB[:].rearrange("p (t m c) -> p t m c", m=4, c=NCH)
        for m in range(NM):
            px = pxp.tile([128, NPOS], F32, space="PSUM", tag="px")
            ms = slice(m * 128, (m + 1) * 128)
            nc.tensor.matmul(px[:], wa_sb[:, ms], xT[0:128, 0:NPOS],
                             start=True, stop=False)
            nc.tensor.matmul(px[:], wa_sb[:, G4 + m * 128: G4 + (m + 1) * 128],
                             xT[0:128, NPOS:2 * NPOS], start=False, stop=False)
            nc.tensor.matmul(px[:], wb_sb[0:E - 256, ms],
                             xT[0:E - 256, 2 * NPOS:3 * NPOS],
                             start=False, stop=False)
            nc.tensor.matmul(px[:], wc_sb[0:1, ms], ones[0:1, :],
                             start=False, stop=True)
            # px columns are (c-major, t-minor); scatter to (t, m, c)
            pxv = px[:].rearrange("p (c t) -> p t c", c=NCH)
            if m < 4:
                dstv = xpvG[:, :, m, :]
            elif m < 12:
                dstv = xpvIF[:, :, m - 4, :]
            else:
                dstv = xpvB[:, :, m - 12, :]
            if m % 2 == 0:
                nc.vector.tensor_copy(dstv, pxv)
            else:
                nc.scalar.copy(dstv, pxv)
        phase_a.close()

        h0c = const.tile([128, NK * NCH], BF16)
        nc.sync.dma_start(h0c[:], h0_d[:, :])
        ones32 = const.tile([128, NK * NCH], F32)
        nc.gpsimd.memset(ones32[:], 1.0)

        psum = ctx.enter_context(tc.tile_pool(name="psum", bufs=2, space="PSUM"))

        c_sb = state.tile([128, NK * NCH], F32)   # (j, c) layout
        nc.sync.dma_start(c_sb[:], c0_d[:, :])
        hT = state.tile([128, S1 * NK * NCH], BF16)  # [(t*NK+j)*NCH+c]

        NB = NK * NCH  # 32

        def hblk(t, j):
            if t < 0:
                return h0c[:, j * NCH:(j + 1) * NCH]
            o = (t * NK + j) * NCH
            return hT[:, o:o + NCH]

        def gate_mms(pg, m0, m1, t, xpt):
            gm = m1 - m0
            nc.tensor.matmul(pg[:], ident[:],
                             xpt[:, t * gm * NCH:(t + 1) * gm * NCH],
                             start=True, stop=False)
            for m in range(m0, m1):
                for j in range(NK):
                    nc.tensor.matmul(
                        pg[:, (m - m0) * NCH:(m - m0 + 1) * NCH],
                        wp[:, j * G4 + m * 128: j * G4 + (m + 1) * 128],
                        hblk(t - 1, j), start=False,
                        stop=(m == m1 - 1 and j == NK - 1),
                    )

        for t in range(S1):
            # gate pre-activations: G = g2 (32), IF = i,f (64), B = o (32)
            pgG = psum.tile([128, 4 * NCH], F32, space="PSUM", tag="pgG")
            pgIF = psum.tile([128, 8 * NCH], F32, space="PSUM", tag="pgIF")
            pgB = psum.tile([128, 4 * NCH], F32, space="PSUM", tag="pgB")
            gate_mms(pgG, 0, 4, t, xpG)
            gate_mms(pgIF, 4, 12, t, xpIF)
            gate_mms(pgB, 12, 16, t, xpB)
            gG = ew.tile([128, 4 * NCH], F32, tag="gG")
            nc.scalar.activation(gG[:], pgG[:], AF.Sigmoid)
            gIF = ew.tile([128, 8 * NCH], F32, tag="gIF")
            nc.scalar.activation(gIF[:], pgIF[:], AF.Sigmoid)
            gB = ew.tile([128, 4 * NCH], F32, tag="gB")
            nc.scalar.activation(gB[:], pgB[:], AF.Sigmoid)
            w4 = ew.tile([128, NB], F32, tag="w4")
            nc.vector.scalar_tensor_tensor(
                out=w4[:], in0=gG[:], scalar=2.0, in1=ones32[:],
                op0=OP.mult, op1=OP.subtract,
            )
            t2 = ew.tile([128, NB], F32, tag="t2")
            nc.vector.tensor_mul(t2[:], gIF[:, NB:2 * NB], c_sb[:])
            t1 = ew.tile([128, NB], F32, tag="t1")
            nc.vector.tensor_mul(t1[:], gIF[:, 0:NB], w4[:])
            nc.vector.tensor_add(c_sb[:], t1[:], t2[:])
            tcc = ew.tile([128, NB], F32, tag="tcc")
            nc.scalar.activation(tcc[:], c_sb[:], AF.Tanh)
            nc.vector.tensor_mul(hT[:, t * NB:(t + 1) * NB], gB[:], tcc[:])

        nc.sync.dma_start(hT_d[:, :], hT[:])
    nc.compile()
    return nc


# --------------------------------------------------------------------------
# L2: feats + chunked CRF scan + backpointer suffix composition
# --------------------------------------------------------------------------
def build_l2():
    nc = _new_nc(8)
    hcat_d = nc.dram_tensor("hcat", [128, 8 * SP2], BF16, kind="ExternalInput").ap()
    wo_d = nc.dram_tensor("woutp", [128, 8 * NT], BF16, kind="ExternalInput").ap()
    bo_d = nc.dram_tensor("bout", [1, NT], BF16, kind="ExternalInput").ap()
    tr_d = nc.dram_tensor("transTp", [32, 32], F32, kind="ExternalInput").ap()
    fv_d = nc.dram_tensor("fvinit", [32, 1], F32, kind="ExternalInput").ap()
    tags_d = nc.dram_tensor("tags", [32, S2], F32, kind="ExternalOutput").ap()
    g_d = nc.dram_tensor("gmat", [32, 32], F32, kind="ExternalOutput").ap()
    term_d = nc.dram_tensor("term", [32, 1], F32, kind="ExternalOutput").ap()

    with tile.TileContext(nc) as tc, ExitStack() as ctx:
        const = ctx.enter_context(tc.tile_pool(name="const", bufs=1))
        st = ctx.enter_context(tc.tile_pool(name="st", bufs=1))

        ident = const.tile([32, 32], F32)
        make_identity(nc, ident[:])
        hcat = const.tile([128, 8 * SP2], BF16)
        nc.sync.dma_start(hcat[:], hcat_d[:, :])
        wo = const.tile([128, 8 * NT], BF16)
        nc.sync.dma_start(wo[:], wo_d[:, :])
        bo = const.tile([1, NT], BF16)
        nc.sync.dma_start(bo[:], bo_d[:, :])
        trT = const.tile([32, 32], F32)
        nc.sync.dma_start(trT[:], tr_d[:, :])
        fvi = const.tile([32, 1], F32)
        nc.sync.dma_start(fvi[:], fv_d[:, :])
        ones = const.tile([1, SP2], BF16)
        nc.gpsimd.memset(ones[:], 1.0)

        # feats^T [20, SP2]
        phase_f = ExitStack()
        psf = phase_f.enter_context(tc.tile_pool(name="psf", bufs=1, space="PSUM"))
        pf = psf.tile([32, SP2], F32, space="PSUM", tag="pf")
        for j in range(8):
            nc.tensor.matmul(
                pf[0:NT, :], wo[:, j * NT:(j + 1) * NT],
                hcat[:, j * SP2:(j + 1) * SP2], start=(j == 0), stop=False,
            )
        nc.tensor.matmul(pf[0:NT, :], bo[0:1, :], ones[0:1, :],
                         start=False, stop=True)
        feats = st.tile([32, SP2], F32)
        nc.gpsimd.memset(feats[:], 0.0)
        nc.scalar.activation(feats[0:NT, :], pf[0:NT, :], AF.Copy)
        phase_f.close()

        # CRF forward scan over S2 steps, with the backpointer one-hot
        # builds and suffix-composition links of each sub-chain emitted as
        # soon as the scan has produced that sub-chain's backpointers (the
        # copies alternate DVE/ACT and fill the scan's engine slack).
        scT = st.tile([32, 32], F32)
        nc.gpsimd.memset(scT[:], 0.0)
        bpt = st.tile([32, 8 * S2], U32)
        schist = st.tile([32, 32 * S2], F32)
        mxhist = st.tile([32, 8 * S2], F32)
        nc.gpsimd.memset(mxhist[:], 0.0)
        iotar = st.tile([32, 32], I32)
        nc.gpsimd.iota(iotar[:], pattern=[[1, 32]], base=0, channel_multiplier=0)
        iotarf = st.tile([32, 32], F32)
        nc.vector.tensor_copy(iotarf[:], iotar[:])
        iotac = st.tile([32, 1], I32)
        nc.gpsimd.iota(iotac[:], pattern=[[0, 1]], base=0, channel_multiplier=1)
        iotacf = st.tile([32, 1], F32)
        nc.vector.tensor_copy(iotacf[:], iotac[:])
        bpf = st.tile([32, S2], F32)
        mall = st.tile([32, S2 * 32], F32)
        scomp = st.tile([32, S2 * 32], F32)
        sc = ctx.enter_context(tc.tile_pool(name="sc", bufs=2))
        phase_l = ExitStack()
        psl = phase_l.enter_context(tc.tile_pool(name="psl", bufs=1, space="PSUM"))
        scur = [None] * NSUB
        nlink = [0]

        nc.vector.tensor_scalar_add(scT[:, 0:NT], trT[:, 0:NT], fvi[:, 0:1])
        mx = None
        for t in range(S2):
            sct = schist[:, 32 * t:32 * (t + 1)]
            nc.vector.transpose(sct, scT[:])
            mx = mxhist[:, 8 * t:8 * t + 8]
            nc.vector.max(mx[0:NT, :], sct[0:NT, 0:NT])
            if t < S2 - 1:
                nc.vector.scalar_tensor_tensor(
                    out=scT[:, 0:NT], in0=trT[:, 0:NT], scalar=mx[:, 0:1],
                    in1=feats[:, t:t + 1].to_broadcast([32, NT]),
                    op0=OP.add, op1=OP.add,
                )
            nc.vector.max_index(
                bpt[0:NT, 8 * t:8 * t + 8], mxhist[0:NT, 8 * t:8 * t + 8],
                schist[0:NT, 32 * t:32 * t + NT],
            )

        # backpointer one-hot maps: mall[p, t*32+n] = (bptr_t[p] == n)
        nc.vector.tensor_copy(
            bpf[0:NT, :],
            bpt[0:NT, :].rearrange("p (t e) -> p t e", e=8)[:, :, 0],
        )
        nc.vector.tensor_tensor(
            out=mall[0:NT, :].rearrange("p (t n) -> p t n", n=32),
            in0=bpf[0:NT, :].rearrange("p (t o) -> p t o", o=1)
                .broadcast_to([NT, S2, 32]),
            in1=iotarf[0:NT, :].rearrange("p (o n) -> p o n", o=1)
                .broadcast_to([NT, S2, 32]),
            op=OP.is_equal,
        )
        for u in range(NSUB):
            nc.scalar.copy(
                scomp[:, 32 * (SUB[u][1] - 1):32 * SUB[u][1]], ident[:])

        # suffix-composition links, round-robin across sub-chains
        for step in range(SUB[0][1] - SUB[0][0]):
            for u in range(NSUB):
                u_lo, u_hi = SUB[u]
                t = u_hi - 1 - step
                srcs = scomp[:, 32 * t:32 * (t + 1)]
                ps = psl.tile([32, 32], F32, space="PSUM", tag=f"ps{u}")
                nc.tensor.matmul(
                    ps[0:32, :], mall[0:NT, t * 32:(t + 1) * 32],
                    srcs[0:NT, :], start=True, stop=True,
                )
                if t > u_lo:
                    dst = scomp[:, 32 * (t - 1):32 * t]
                else:
                    nxt = sc.tile([32, 32], F32, tag=f"sc{u}", name=f"sloc{u}")
                    scur[u] = nxt
                    dst = nxt[:]
                if nlink[0] % 2 == 0:
                    nc.vector.tensor_copy(dst, ps[:])
                else:
                    nc.scalar.copy(dst, ps[:])
                nlink[0] += 1
        phase_l.close()

        # terminal one-hot (valid on core 7)
        phase_t = ExitStack()
        pst = phase_t.enter_context(tc.tile_pool(name="pst", bufs=1, space="PSUM"))
        term = st.tile([32, 1], F32)
        nc.gpsimd.memset(term[:], NEG)
        nc.vector.scalar_tensor_tensor(
            out=term[0:NT, :], in0=trT[0:NT, STOP:STOP + 1],
            scalar=mx[0:NT, 0:1], in1=feats[0:NT, S2 - 1:S2],
            op0=OP.add, op1=OP.add,
        )
        t32 = st.tile([32, 32], F32)
        nc.gpsimd.memset(t32[:], NEG)
        nc.vector.tensor_copy(t32[:, 0:1], term[:])
        tT = st.tile([32, 32], F32)
        nc.vector.transpose(tT[:], t32[:])
        mxt = st.tile([32, 8], F32)
        nc.vector.max(mxt[0:1, :], tT[0:1, 0:NT])
        onesf = st.tile([1, NT], F32)
        nc.gpsimd.memset(onesf[:], 1.0)
        pmx = pst.tile([32, 1], F32, space="PSUM", tag="pmx")
        nc.tensor.matmul(pmx[0:NT, :], onesf[0:1, 0:NT], mxt[0:1, 0:1],
                         start=True, stop=True)
        mxb = st.tile([32, 1], F32)
        nc.vector.tensor_copy(mxb[0:NT, :], pmx[0:NT, :])
        termOH = st.tile([32, 1], F32)
        nc.gpsimd.memset(termOH[:], 0.0)
        nc.vector.tensor_scalar(
            termOH[0:NT, :], term[0:NT, :], mxb[0:NT, 0:1], None, OP.is_equal,
        )
        nc.sync.dma_start(term_d[:, :], termOH[:])
        phase_t.close()

        # cross-chain products C_u = S^u_loc . C_{u+1}, C_NSUB = identity
        psc = ctx.enter_context(tc.tile_pool(name="psc", bufs=1, space="PSUM"))
        cmats = [None] * (NSUB + 1)
        cmats[NSUB] = ident
        for u in range(NSUB - 1, 0, -1):
            sT = st.tile([32, 32], F32, name=f"sT{u}")
            nc.vector.transpose(sT[:], scur[u][:])
            pc = psc.tile([32, 32], F32, space="PSUM", tag="pc")
            nc.tensor.matmul(pc[0:32, :], sT[0:32, :], cmats[u + 1][0:32, :],
                             start=True, stop=True)
            cm = st.tile([32, 32], F32, name=f"cm{u}")
            nc.vector.tensor_copy(cm[:], pc[:])
            cmats[u] = cm

        # G matrix: full suffix at t = W2 - 1: G = S^{u*}_{W2-1} . C_{u*+1}
        ustar = next(u for u in range(NSUB)
                     if SUB[u][0] <= W2 - 1 < SUB[u][1])
        sT31 = st.tile([32, 32], F32)
        nc.vector.transpose(sT31[:], scomp[:, 32 * (W2 - 1):32 * W2])
        pg = psc.tile([32, 32], F32, space="PSUM", tag="pg")
        nc.tensor.matmul(pg[0:32, :], sT31[0:32, :], cmats[ustar + 1][0:32, :],
                         start=True, stop=True)
        gsb = st.tile([32, 32], F32)
        nc.vector.tensor_copy(gsb[:], pg[:])
        nc.sync.dma_start(g_d[:, :], gsb[:])

        # tag columns: w_t = S^u_t^T iota, then per sub-chain join
        ptw = psc.tile([32, S2], F32, space="PSUM", tag="ptw")
        for t in range(S2):
            nc.tensor.matmul(
                ptw[0:32, t:t + 1], scomp[0:NT, 32 * t:32 * t + 32],
                iotacf[0:NT, 0:1], start=True, stop=True,
            )
        wtags = st.tile([32, S2], F32)
        nc.vector.tensor_copy(wtags[:], ptw[:])
        ptag = psc.tile([32, S2], F32, space="PSUM", tag="ptag")
        for u in range(NSUB):
            us = slice(SUB[u][0], SUB[u][1])
            nc.tensor.matmul(ptag[0:32, us], cmats[u + 1][0:NT, :],
                             wtags[0:NT, us], start=True, stop=True)
        tags = st.tile([32, S2], F32)
        nc.vector.tensor_copy(tags[:], ptag[:])
        nc.sync.dma_start(tags_d[:, :], tags[:])
    nc.compile()
    return nc


# --------------------------------------------------------------------------
# L3: cross-chunk stitch
# --------------------------------------------------------------------------
def build_l3():
    nc = _new_nc(1)
    NIN = 8 * S2 + 8 * 32 + 1
    in_d = nc.dram_tensor("stitchin", [32, NIN], F32, kind="ExternalInput").ap()
    path_d = nc.dram_tensor("path", [1, L], I32, kind="ExternalOutput").ap()

    with tile.TileContext(nc) as tc, ExitStack() as ctx:
        st = ctx.enter_context(tc.tile_pool(name="st", bufs=1))
        psum = ctx.enter_context(tc.tile_pool(name="psum", bufs=2, space="PSUM"))

        allin = st.tile([32, NIN], F32)
        nc.sync.dma_start(allin[:], in_d[:, :])
        tags = allin[:, 0:8 * S2]
        gt = allin[:, 8 * S2:8 * S2 + 8 * 32]
        term = allin[:, NIN - 1:NIN]

        # chunk-end one-hots: v[7] = term; v[c-1] = G_c . v[c] via lhsT=G_c^T
        vsb = st.tile([32, 8], F32)
        nc.vector.tensor_copy(vsb[:, 7:8], term)
        for c in range(7, 0, -1):
            pv = psum.tile([32, 1], F32, space="PSUM", tag="pv")
            nc.tensor.matmul(pv[0:32, :],
                             allin[0:NT, 8 * S2 + 32 * c:8 * S2 + 32 * c + 32],
                             vsb[0:NT, c:c + 1], start=True, stop=True)
            nc.vector.tensor_copy(vsb[:, c - 1:c], pv[:])

        pp = psum.tile([32, L], F32, space="PSUM", tag="pp")
        for c in range(8):
            base = c * S2 + (0 if c == 0 else W2)
            lo = 0 if c == 0 else S2 + (c - 1) * CL2
            ln = S2 if c == 0 else CL2
            nc.tensor.matmul(pp[0:1, lo:lo + ln], vsb[0:NT, c:c + 1],
                             allin[0:NT, base:base + ln], start=True, stop=True)
        path_sb = st.tile([1, L], I32)
        nc.vector.tensor_copy(path_sb[:], pp[0:1, :])
        nc.sync.dma_start(path_d[:, :], path_sb[:])
    nc.compile()
    return nc


# --------------------------------------------------------------------------
# host glue
# --------------------------------------------------------------------------
def _get(name, builder):
    if name not in _CACHE:
        _CACHE[name] = builder()
    return _CACHE[name]


def _prep_l1_core(sentence, embed_table, wih, bih, bhh, whh, h0, c0, k, reverse):
    """Inputs for one L1 core: direction weights + 8 chain windows."""
    import ml_dtypes
    s = np.asarray(sentence, np.int64)
    if reverse:
        s = s[::-1]
    # chain c on core k handles chunk q = k*NCH + c, window [CL1*q, CL1*q+S1)
    pos = np.zeros(NCH * SP1, np.int64)
    for c in range(NCH):
        q = k * NCH + c
        s0 = CL1 * q
        pos[c * SP1:c * SP1 + S1] = np.arange(s0, s0 + S1)
    inds = s[pos]
    ins = {
        "emb": np.asarray(embed_table, np.float32),
        "sent": np.ascontiguousarray(
            np.asarray(inds, np.int32).reshape(NG, 128).T),
    }
    w = np.asarray(wih, np.float32)[_PERM].copy()    # [2048, 300] g,i,f,o
    b = (np.asarray(bih, np.float32) + np.asarray(bhh, np.float32))[_PERM].copy()
    w[0:H] *= 2.0   # g-gate x2 for tanh(g) = 2*sigmoid(2g)-1
    b[0:H] *= 2.0
    wT = np.ascontiguousarray(w.T)                   # [300, 2048]
    ins["wA"] = np.ascontiguousarray(
        np.concatenate([wT[0:128], wT[128:256]], axis=1)).astype(ml_dtypes.bfloat16)
    ins["wB"] = np.ascontiguousarray(wT[256:300]).astype(ml_dtypes.bfloat16)
    ins["wC"] = np.ascontiguousarray(b[None, :]).astype(ml_dtypes.bfloat16)
    wh = np.asarray(whh, np.float32)[_PERM].copy()   # [2048, 512]
    wh[0:H] *= 2.0
    whT = np.ascontiguousarray(wh.T)                 # [512, 2048]
    ins["wpack"] = np.ascontiguousarray(
        whT.reshape(NK, 128, G4).transpose(1, 0, 2).reshape(128, NK * G4)
    ).astype(ml_dtypes.bfloat16)
    h0c = np.zeros((128, NK * NCH), np.float32)
    c0c = np.zeros((128, NK * NCH), np.float32)
    if k == 0:
        # chain 0 is chunk 0: true initial state; (j, c) layout
        h0j = np.asarray(h0, np.float32).reshape(NK, 128).T  # [128, NK]
        c0j = np.asarray(c0, np.float32).reshape(NK, 128).T
        for j in range(NK):
            h0c[:, j * NCH] = h0j[:, j]
            c0c[:, j * NCH] = c0j[:, j]
    ins["h0c"] = h0c.astype(ml_dtypes.bfloat16)
    ins["c0c"] = c0c
    return ins


def _assemble_h(results, reverse):
    """Assemble full h history [128, NK, L] (time order) for one direction."""
    hfull = np.zeros((128, NK, L), np.float32)
    for k in range(4):
        out = np.asarray(results[k], np.float32).reshape(128, S1, NK, NCH)
        for c in range(NCH):
            q = k * NCH + c
            wq = 0 if q == 0 else W1
            lo = CL1 * q + wq
            hi = CL1 * q + S1
            hfull[:, :, lo:hi] = np.moveaxis(out[:, wq:, :, c], 1, 2)
    if reverse:
        hfull = hfull[:, :, ::-1]
    return hfull


def _prep_l2_core(hf, hb, wout, bout, transitions, c2):
    import ml_dtypes
    w0 = CL2 * c2
    win = slice(w0, w0 + S2)
    hcat = np.zeros((128, 8 * SP2), np.float32)
    for j in range(NK):
        hcat[:, j * SP2:j * SP2 + S2] = hf[:, j, win]
        hcat[:, (NK + j) * SP2:(NK + j) * SP2 + S2] = hb[:, j, win]
    woT = np.ascontiguousarray(np.asarray(wout, np.float32).T)  # [1024, 20]
    wop = np.ascontiguousarray(
        np.concatenate([woT[j * 128:(j + 1) * 128] for j in range(8)], axis=1))
    trTp = np.zeros((32, 32), np.float32)
    trTp[0:NT, 0:NT] = np.asarray(transitions, np.float32).T
    fvi = np.zeros((32, 1), np.float32)
    if c2 == 0:
        fvi[0:NT, 0] = NEG
        fvi[START, 0] = 0.0
    return {
        "hcat": hcat.astype(ml_dtypes.bfloat16),
        "woutp": wop.astype(ml_dtypes.bfloat16),
        "bout": np.ascontiguousarray(
            np.asarray(bout, np.float32)[None, :]).astype(ml_dtypes.bfloat16),
        "transTp": trTp,
        "fvinit": fvi,
    }


def kernel(sentence, embed_table, w_ih_f, w_hh_f, b_ih_f, b_hh_f,
           w_ih_b, w_hh_b, b_ih_b, b_hh_b, h0, c0, w_out, b_out, transitions):
    h0 = np.asarray(h0, np.float32)
    c0 = np.asarray(c0, np.float32)

    # ---- L1: 8 cores, 32 LSTM chunks per direction
    nc1 = _get("l1", build_l1)
    ins1 = []
    for core in range(8):
        rev = core >= 4
        k = core % 4
        if rev:
            ins1.append(_prep_l1_core(sentence, embed_table, w_ih_b, b_ih_b,
                                      b_hh_b, w_hh_b, h0[1], c0[1], k, True))
        else:
            ins1.append(_prep_l1_core(sentence, embed_table, w_ih_f, b_ih_f,
                                      b_hh_f, w_hh_f, h0[0], c0[0], k, False))
    r1 = run_bass_kernel_spmd(nc1, ins1, core_ids=list(range(8))).results
    hf = _assemble_h([r1[k]["hT_out"] for k in range(4)], False)
    hb = _assemble_h([r1[4 + k]["hT_out"] for k in range(4)], True)

    # ---- L2: 8 cores, chunked CRF scan + composition
    nc2 = _get("l2", build_l2)
    ins2 = [_prep_l2_core(hf, hb, w_out, b_out, transitions, c2)
            for c2 in range(8)]
    r2 = run_bass_kernel_spmd(nc2, ins2, core_ids=list(range(8))).results

    # ---- L3: stitch
    nc3 = _get("l3", build_l3)
    tagsall = np.concatenate([np.asarray(r2[c]["tags"], np.float32)
                              for c in range(8)], axis=1)
    gtall = np.concatenate(
        [np.ascontiguousarray(np.asarray(r2[c]["gmat"], np.float32).T)
         for c in range(8)], axis=1)
    ins3 = {"stitchin": np.ascontiguousarray(np.concatenate(
        [tagsall, gtall, np.asarray(r2[7]["term"], np.float32)], axis=1))}
    r3 = run_bass_kernel_spmd(nc3, [ins3], core_ids=[0]).results[0]
    return np.ascontiguousarray(r3["path"].reshape(L)).astype(np.int32)


# revision 29
# speedup vs baseline: 1.0007x; 1.0007x over previous
"""BiLSTM-CRF Trainium2 kernel (Bass/Tile), three SPMD launches.

The 512-step LSTM recurrence and the 512-step CRF Viterbi scan are both
latency-chain bound on-chip (each step is a serial PE->ACT->DVE chain).
Both recurrences are exponentially forgetting, so they are chunked across
all 8 cores with warm-up prefixes that converge to the sequential
trajectory far below the (already path-exact) bf16 noise floor:

  L1 (8 cores): per direction, 32 LSTM chunks of 15 steps with a 32-step
      zero-state warm-up (chunk 0 starts from the true h0/c0 and is 47
      long). Each core runs its 8 chunks in lockstep: the chain index is
      a free-axis column, so one Ldweights+Matmult pair serves all 8
      chains ([128,8] moving operand) and the sigmoid/tanh/elementwise
      ops batch across chains. tanh(g) is computed as 2*sigmoid(2g)-1
      (g weights pre-scaled by 2 on the host, exact in bf16), so a step
      is: one sigmoid over i,f,g, one sigmoid over o, 4 DVE ops, one
      tanh, one h-multiply.
  L2 (8 cores): CRF Viterbi scan in 8 chunks of 62 steps with a 16-step
      zero-state warm-up (chunk 0 is 78 long, exact START init). Each
      core then composes its chunk's backpointer one-hot maps into
      suffix products on PE (6 concurrent sub-chains), emitting per-step
      "tag given chunk-end-tag" columns and the whole-chunk map G.
  L3 (1 core): chains the 8 chunk maps G to pick each chunk's end tag,
      then selects each chunk's tag columns with one matmul per chunk.

Host work is sharding glue: dtype casts, weight re-layout (incl. the x2
g-gate scaling), window slicing, time reversal for the backward
direction, and output concat.
"""

import numpy as np
from contextlib import ExitStack

import concourse.bass as bass
import concourse.tile as tile
from concourse import bacc, mybir
from concourse.bass_utils import run_bass_kernel_spmd
from concourse.masks import make_identity

F32 = mybir.dt.float32
I32 = mybir.dt.int32
U32 = mybir.dt.uint32
BF16 = mybir.dt.bfloat16
AF = mybir.ActivationFunctionType
OP = mybir.AluOpType

V, E, H, L = 100000, 300, 512, 512
NT, START, STOP, NEG = 20, 18, 19, -10000.0
G4 = 4 * H   # 2048
NM = G4 // 128  # 16 gate column-chunks
NK = H // 128   # 4 h row-chunks

# --- L1 chunking: 32 chunks per direction, 4 cores x 8 lockstep chains ---
NCH = 8       # chains per core (lockstep, chain = free-axis column)
W1 = 32       # LSTM warm-up steps
S1 = 47       # steps per chain (= W1 + CL1 = chunk-0 length)
CL1 = S1 - W1  # 15
SP1 = 48      # padded per-chain stride in the gathered window (8*48=384)
NG = 3        # gather tiles of 128 positions
NPOS = NG * 128  # 384
assert S1 + 31 * CL1 == L and NCH * SP1 == NPOS

# --- L2 chunking: 8 chunks, 1 per core ---
W2 = 16
S2 = 78       # scan steps per core (= W2 + CL2 = chunk-0 length)
CL2 = S2 - W2  # 62
SP2 = 80      # padded hcat window stride
NSUB = 6      # composition sub-chains
SUB = [(13 * u, 13 * (u + 1)) for u in range(NSUB)]
assert S2 + 7 * CL2 == L and SUB[-1][1] == S2

# gate row order used on-chip: g, i, f, o (sigmoid over g2 fires first)
_PERM = np.concatenate([
    np.arange(2 * H, 3 * H),  # g
    np.arange(0, H),          # i
    np.arange(H, 2 * H),      # f
    np.arange(3 * H, 4 * H),  # o
])

_CACHE: dict = {}


def _new_nc(num_devices):
    return bacc.Bacc(
        "TRN2", target_bir_lowering=False, debug=False, num_devices=num_devices
    )


# --------------------------------------------------------------------------
# L1: gather + input projection + 8 lockstep LSTM chunk recurrences
# --------------------------------------------------------------------------
def build_l1():
    nc = _new_nc(8)
    emb_d = nc.dram_tensor("emb", [V, E], F32, kind="ExternalInput").ap()
    sent_d = nc.dram_tensor("sent", [128, NG], I32, kind="ExternalInput").ap()
    wA_d = nc.dram_tensor("wA", [128, 2 * G4], BF16, kind="ExternalInput").ap()
    wB_d = nc.dram_tensor("wB", [E - 256, G4], BF16, kind="ExternalInput").ap()
    wC_d = nc.dram_tensor("wC", [1, G4], BF16, kind="ExternalInput").ap()
    wp_d = nc.dram_tensor("wpack", [128, NK * G4], BF16, kind="ExternalInput").ap()
    h0_d = nc.dram_tensor("h0c", [128, NK * NCH], BF16, kind="ExternalInput").ap()
    c0_d = nc.dram_tensor("c0c", [128, NK * NCH], F32, kind="ExternalInput").ap()
    hT_d = nc.dram_tensor("hT_out", [128, S1 * NK * NCH], BF16,
                          kind="ExternalOutput").ap()

    with tile.TileContext(nc) as tc, ExitStack() as ctx:
        const = ctx.enter_context(tc.tile_pool(name="const", bufs=1))
        state = ctx.enter_context(tc.tile_pool(name="state", bufs=1))
        ew = ctx.enter_context(tc.tile_pool(name="ew", bufs=3))

        ident = const.tile([128, 128], F32)
        make_identity(nc, ident[:])
        # x-projection, one tile per gate group so the recurrence can
        # start as soon as the g-group columns are written:
        # xpG (g: m 0..4), xpIF (i,f: m 4..12), xpB (o: m 12..16),
        # layout [128, (t*gm + (m-m0))*NCH + c]
        xpG = const.tile([128, SP1 * 4 * NCH], F32)
        xpIF = const.tile([128, SP1 * 8 * NCH], F32)
        xpB = const.tile([128, SP1 * 4 * NCH], F32)

        # --- phase A: gather + transpose + input projection ---
        # (index DMA first so the gathers don't queue behind the weights)
        phase_a = ExitStack()
        pxp = phase_a.enter_context(tc.tile_pool(name="pxp", bufs=2, space="PSUM"))
        ptp = phase_a.enter_context(tc.tile_pool(name="ptp", bufs=1, space="PSUM"))
        ones = const.tile([1, NPOS], BF16)
        nc.gpsimd.memset(ones[:], 1.0)
        idx = const.tile([128, NG], I32)
        nc.sync.dma_start(idx[:], sent_d[:, :])
        xg = []
        for g in range(NG):
            t = const.tile([128, E], F32, tag=f"xg{g}", name=f"xg{g}")
            nc.gpsimd.indirect_dma_start(
                out=t[:], out_offset=None, in_=emb_d[:, :],
                in_offset=bass.IndirectOffsetOnAxis(ap=idx[:, g:g + 1], axis=0),
            )
            xg.append(t)
        ecs = [128, 128, E - 256]
        xT = const.tile([128, 3 * NPOS], BF16)
        for e in range(3):
            e0 = sum(ecs[:e])
            for g in range(NG):
                pt = ptp.tile([128, 128], F32, space="PSUM", tag="pt")
                nc.tensor.transpose(
                    out=pt[0:ecs[e], :], in_=xg[g][:, e0:e0 + ecs[e]],
                    identity=ident[:],
                )
                nc.vector.tensor_copy(
                    xT[0:ecs[e], e * NPOS + g * 128: e * NPOS + (g + 1) * 128],
                    pt[0:ecs[e], :],
                )
        # weight loads go through the same (Pool/SWDGE) queue as the
        # gathers, AFTER them, so the gathers aren't stuck behind 3 MB of
        # weights on the DMA engines. rowi artificially depends on idx so
        # the scheduler cannot hoist the weight loads above the gathers.
        rowi0 = const.tile([128, 1], I32)
        nc.gpsimd.iota(rowi0[:], pattern=[[0, 1]], base=0, channel_multiplier=1)
        rowi = const.tile([128, 1], I32)
        nc.vector.scalar_tensor_tensor(
            out=rowi[:], in0=idx[:, 0:1], scalar=0, in1=rowi0[:],
            op0=OP.mult, op1=OP.add,
        )
        wa_sb = const.tile([128, 2 * G4], BF16)
        nc.gpsimd.indirect_dma_start(
            out=wa_sb[:], out_offset=None, in_=wA_d[:, :],
            in_offset=bass.IndirectOffsetOnAxis(ap=rowi[:, 0:1], axis=0))
        wb_sb = const.tile([E - 256, G4], BF16)
        nc.gpsimd.indirect_dma_start(
            out=wb_sb[:], out_offset=None, in_=wB_d[:, :],
            in_offset=bass.IndirectOffsetOnAxis(ap=rowi[0:E - 256, 0:1], axis=0))
        wc_sb = const.tile([1, G4], BF16)
        nc.sync.dma_start(wc_sb[:], wC_d[:, :])
        wp = const.tile([128, NK * G4], BF16)
        nc.gpsimd.indirect_dma_start(
            out=wp[:], out_offset=None, in_=wp_d[:, :],
            in_offset=bass.IndirectOffsetOnAxis(ap=rowi[:, 0:1], axis=0))
        xpvG = xpG[:].rearrange("p (t m c) -> p t m c", m=4, c=NCH)
        xpvIF = xpIF[:].rearrange("p (t m c) -> p t m c", m=8, c=NCH)
        xpvB = xpB[:].rearrange("p (t m c) -> p t m c", m=4, c=NCH)
        for m in range(NM):
            px = pxp.tile([128, NPOS], F32, space="PSUM", tag="px")
            ms = slice(m * 128, (m + 1) * 128)
            nc.tensor.matmul(px[:], wa_sb[:, ms], xT[0:128, 0:NPOS],
                             start=True, stop=False)
            nc.tensor.matmul(px[:], wa_sb[:, G4 + m * 128: G4 + (m + 1) * 128],
                             xT[0:128, NPOS:2 * NPOS], start=False, stop=False)
            nc.tensor.matmul(px[:], wb_sb[0:E - 256, ms],
                             xT[0:E - 256, 2 * NPOS:3 * NPOS],
                             start=False, stop=False)
            nc.tensor.matmul(px[:], wc_sb[0:1, ms], ones[0:1, :],
                             start=False, stop=True)
            # px columns are (c-major, t-minor); scatter to (t, m, c)
            pxv = px[:].rearrange("p (c t) -> p t c", c=NCH)
            if m < 4:
                dstv = xpvG[:, :, m, :]
            elif m < 12:
                dstv = xpvIF[:, :, m - 4, :]
            else:
                dstv = xpvB[:, :, m - 12, :]
            if m % 2 == 0:
                nc.vector.tensor_copy(dstv, pxv)
            else:
                nc.scalar.copy(dstv, pxv)
        phase_a.close()

        h0c = const.tile([128, NK * NCH], BF16)
        nc.sync.dma_start(h0c[:], h0_d[:, :])
        ones32 = const.tile([128, NK * NCH], F32)
        nc.gpsimd.memset(ones32[:], 1.0)

        psum = ctx.enter_context(tc.tile_pool(name="psum", bufs=2, space="PSUM"))

        c_sb = state.tile([128, NK * NCH], F32)   # (j, c) layout
        nc.sync.dma_start(c_sb[:], c0_d[:, :])
        hT = state.tile([128, S1 * NK * NCH], BF16)  # [(t*NK+j)*NCH+c]

        NB = NK * NCH  # 32

        def hblk(t, j):
            if t < 0:
                return h0c[:, j * NCH:(j + 1) * NCH]
            o = (t * NK + j) * NCH
            return hT[:, o:o + NCH]

        def gate_mms(pg, m0, m1, t, xpt):
            gm = m1 - m0
            nc.tensor.matmul(pg[:], ident[:],
                             xpt[:, t * gm * NCH:(t + 1) * gm * NCH],
                             start=True, stop=False)
            for m in range(m0, m1):
                for j in range(NK):
                    nc.tensor.matmul(
                        pg[:, (m - m0) * NCH:(m - m0 + 1) * NCH],
                        wp[:, j * G4 + m * 128: j * G4 + (m + 1) * 128],
                        hblk(t - 1, j), start=False,
                        stop=(m == m1 - 1 and j == NK - 1),
                    )

        for t in range(S1):
            # gate pre-activations: G = g2 (32), IF = i,f (64), B = o (32)
            pgG = psum.tile([128, 4 * NCH], F32, space="PSUM", tag="pgG")
            pgIF = psum.tile([128, 8 * NCH], F32, space="PSUM", tag="pgIF")
            pgB = psum.tile([128, 4 * NCH], F32, space="PSUM", tag="pgB")
            gate_mms(pgG, 0, 4, t, xpG)
            gate_mms(pgIF, 4, 12, t, xpIF)
            gate_mms(pgB, 12, 16, t, xpB)
            gG = ew.tile([128, 4 * NCH], F32, tag="gG")
            nc.scalar.activation(gG[:], pgG[:], AF.Sigmoid)
            gIF = ew.tile([128, 8 * NCH], F32, tag="gIF")
            nc.scalar.activation(gIF[:], pgIF[:], AF.Sigmoid)
            gB = ew.tile([128, 4 * NCH], F32, tag="gB")
            nc.scalar.activation(gB[:], pgB[:], AF.Sigmoid)
            w4 = ew.tile([128, NB], F32, tag="w4")
            nc.vector.scalar_tensor_tensor(
                out=w4[:], in0=gG[:], scalar=2.0, in1=ones32[:],
                op0=OP.mult, op1=OP.subtract,
            )
            t2 = ew.tile([128, NB], F32, tag="t2")
            nc.vector.tensor_mul(t2[:], gIF[:, NB:2 * NB], c_sb[:])
            t1 = ew.tile([128, NB], F32, tag="t1")
            nc.vector.tensor_mul(t1[:], gIF[:, 0:NB], w4[:])
            nc.vector.tensor_add(c_sb[:], t1[:], t2[:])
            tcc = ew.tile([128, NB], F32, tag="tcc")
            nc.scalar.activation(tcc[:], c_sb[:], AF.Tanh)
            nc.vector.tensor_mul(hT[:, t * NB:(t + 1) * NB], gB[:], tcc[:])

        nc.sync.dma_start(hT_d[:, :], hT[:])
    nc.compile()
    return nc


# --------------------------------------------------------------------------
# L2: feats + chunked CRF scan + backpointer suffix composition
# --------------------------------------------------------------------------
def build_l2():
    nc = _new_nc(8)
    hcat_d = nc.dram_tensor("hcat", [128, 8 * SP2], BF16, kind="ExternalInput").ap()
    wo_d = nc.dram_tensor("woutp", [128, 8 * NT], BF16, kind="ExternalInput").ap()
    bo_d = nc.dram_tensor("bout", [1, NT], BF16, kind="ExternalInput").ap()
    tr_d = nc.dram_tensor("transTp", [32, 32], F32, kind="ExternalInput").ap()
    fv_d = nc.dram_tensor("fvinit", [32, 1], F32, kind="ExternalInput").ap()
    tags_d = nc.dram_tensor("tags", [32, S2], F32, kind="ExternalOutput").ap()
    g_d = nc.dram_tensor("gmat", [32, 32], F32, kind="ExternalOutput").ap()
    term_d = nc.dram_tensor("term", [32, 1], F32, kind="ExternalOutput").ap()

    with tile.TileContext(nc) as tc, ExitStack() as ctx:
        const = ctx.enter_context(tc.tile_pool(name="const", bufs=1))
        st = ctx.enter_context(tc.tile_pool(name="st", bufs=1))

        ident = const.tile([32, 32], F32)
        make_identity(nc, ident[:])
        hcat = const.tile([128, 8 * SP2], BF16)
        nc.sync.dma_start(hcat[:], hcat_d[:, :])
        wo = const.tile([128, 8 * NT], BF16)
        nc.sync.dma_start(wo[:], wo_d[:, :])
        bo = const.tile([1, NT], BF16)
        nc.sync.dma_start(bo[:], bo_d[:, :])
        trT = const.tile([32, 32], F32)
        nc.sync.dma_start(trT[:], tr_d[:, :])
        fvi = const.tile([32, 1], F32)
        nc.sync.dma_start(fvi[:], fv_d[:, :])
        ones = const.tile([1, SP2], BF16)
        nc.gpsimd.memset(ones[:], 1.0)

        # feats^T [20, SP2]
        phase_f = ExitStack()
        psf = phase_f.enter_context(tc.tile_pool(name="psf", bufs=1, space="PSUM"))
        pf = psf.tile([32, SP2], F32, space="PSUM", tag="pf")
        for j in range(8):
            nc.tensor.matmul(
                pf[0:NT, :], wo[:, j * NT:(j + 1) * NT],
                hcat[:, j * SP2:(j + 1) * SP2], start=(j == 0), stop=False,
            )
        nc.tensor.matmul(pf[0:NT, :], bo[0:1, :], ones[0:1, :],
                         start=False, stop=True)
        feats = st.tile([32, SP2], F32)
        nc.gpsimd.memset(feats[:], 0.0)
        nc.scalar.activation(feats[0:NT, :], pf[0:NT, :], AF.Copy)
        phase_f.close()

        # CRF forward scan over S2 steps, with the backpointer one-hot
        # builds and suffix-composition links of each sub-chain emitted as
        # soon as the scan has produced that sub-chain's backpointers (the
        # copies alternate DVE/ACT and fill the scan's engine slack).
        scT = st.tile([32, 32], F32)
        nc.gpsimd.memset(scT[:], 0.0)
        bpt = st.tile([32, 8 * S2], U32)
        schist = st.tile([32, 32 * S2], F32)
        mxhist = st.tile([32, 8 * S2], F32)
        nc.gpsimd.memset(mxhist[:], 0.0)
        iotar = st.tile([32, 32], I32)
        nc.gpsimd.iota(iotar[:], pattern=[[1, 32]], base=0, channel_multiplier=0)
        iotarf = st.tile([32, 32], F32)
        nc.vector.tensor_copy(iotarf[:], iotar[:])
        iotac = st.tile([32, 1], I32)
        nc.gpsimd.iota(iotac[:], pattern=[[0, 1]], base=0, channel_multiplier=1)
        iotacf = st.tile([32, 1], F32)
        nc.vector.tensor_copy(iotacf[:], iotac[:])
        bpf = st.tile([32, S2], F32)
        mall = st.tile([32, S2 * 32], F32)
        scomp = st.tile([32, S2 * 32], F32)
        sc = ctx.enter_context(tc.tile_pool(name="sc", bufs=2))
        phase_l = ExitStack()
        psl = phase_l.enter_context(tc.tile_pool(name="psl", bufs=1, space="PSUM"))
        scur = [None] * NSUB
        nlink = [0]

        nc.vector.tensor_scalar_add(scT[:, 0:NT], trT[:, 0:NT], fvi[:, 0:1])
        mx = None
        for t in range(S2):
            sct = schist[:, 32 * t:32 * (t + 1)]
            nc.vector.transpose(sct, scT[:])
            mx = mxhist[:, 8 * t:8 * t + 8]
            nc.vector.max(mx[0:NT, :], sct[0:NT, 0:NT])
            if t < S2 - 1:
                nc.vector.scalar_tensor_tensor(
                    out=scT[:, 0:NT], in0=trT[:, 0:NT], scalar=mx[:, 0:1],
                    in1=feats[:, t:t + 1].to_broadcast([32, NT]),
                    op0=OP.add, op1=OP.add,
                )
            nc.vector.max_index(
                bpt[0:NT, 8 * t:8 * t + 8], mxhist[0:NT, 8 * t:8 * t + 8],
                schist[0:NT, 32 * t:32 * t + NT],
            )

        # backpointer one-hot maps: mall[p, t*32+n] = (bptr_t[p] == n)
        nc.vector.tensor_copy(
            bpf[0:NT, :],
            bpt[0:NT, :].rearrange("p (t e) -> p t e", e=8)[:, :, 0],
        )
        nc.vector.tensor_tensor(
            out=mall[0:NT, :].rearrange("p (t n) -> p t n", n=32),
            in0=bpf[0:NT, :].rearrange("p (t o) -> p t o", o=1)
                .broadcast_to([NT, S2, 32]),
            in1=iotarf[0:NT, :].rearrange("p (o n) -> p o n", o=1)
                .broadcast_to([NT, S2, 32]),
            op=OP.is_equal,
        )
        for u in range(NSUB):
            nc.scalar.copy(
                scomp[:, 32 * (SUB[u][1] - 1):32 * SUB[u][1]], ident[:])

        # suffix-composition links, round-robin across sub-chains
        for step in range(SUB[0][1] - SUB[0][0]):
            for u in range(NSUB):
                u_lo, u_hi = SUB[u]
                t = u_hi - 1 - step
                srcs = scomp[:, 32 * t:32 * (t + 1)]
                ps = psl.tile([32, 32], F32, space="PSUM", tag=f"ps{u}")
                nc.tensor.matmul(
                    ps[0:32, :], mall[0:NT, t * 32:(t + 1) * 32],
                    srcs[0:NT, :], start=True, stop=True,
                )
                if t > u_lo:
                    dst = scomp[:, 32 * (t - 1):32 * t]
                else:
                    nxt = sc.tile([32, 32], F32, tag=f"sc{u}", name=f"sloc{u}")
                    scur[u] = nxt
                    dst = nxt[:]
                if nlink[0] % 2 == 0:
                    nc.vector.tensor_copy(dst, ps[:])
                else:
                    nc.scalar.copy(dst, ps[:])
                nlink[0] += 1
        phase_l.close()

        # terminal one-hot (valid on core 7)
        phase_t = ExitStack()
        pst = phase_t.enter_context(tc.tile_pool(name="pst", bufs=1, space="PSUM"))
        term = st.tile([32, 1], F32)
        nc.gpsimd.memset(term[:], NEG)
        nc.vector.scalar_tensor_tensor(
            out=term[0:NT, :], in0=trT[0:NT, STOP:STOP + 1],
            scalar=mx[0:NT, 0:1], in1=feats[0:NT, S2 - 1:S2],
            op0=OP.add, op1=OP.add,
        )
        t32 = st.tile([32, 32], F32)
        nc.gpsimd.memset(t32[:], NEG)
        nc.vector.tensor_copy(t32[:, 0:1], term[:])
        tT = st.tile([32, 32], F32)
        nc.vector.transpose(tT[:], t32[:])
        mxt = st.tile([32, 8], F32)
        nc.vector.max(mxt[0:1, :], tT[0:1, 0:NT])
        onesf = st.tile([1, NT], F32)
        nc.gpsimd.memset(onesf[:], 1.0)
        pmx = pst.tile([32, 1], F32, space="PSUM", tag="pmx")
        nc.tensor.matmul(pmx[0:NT, :], onesf[0:1, 0:NT], mxt[0:1, 0:1],
                         start=True, stop=True)
        mxb = st.tile([32, 1], F32)
        nc.vector.tensor_copy(mxb[0:NT, :], pmx[0:NT, :])
        termOH = st.tile([32, 1], F32)
        nc.gpsimd.memset(termOH[:], 0.0)
        nc.vector.tensor_scalar(
            termOH[0:NT, :], term[0:NT, :], mxb[0:NT, 0:1], None, OP.is_equal,
        )
        nc.sync.dma_start(term_d[:, :], termOH[:])
        phase_t.close()

        # cross-chain products C_u = S^u_loc . C_{u+1}, C_NSUB = identity
        psc = ctx.enter_context(tc.tile_pool(name="psc", bufs=1, space="PSUM"))
        cmats = [None] * (NSUB + 1)
        cmats[NSUB] = ident
        for u in range(NSUB - 1, 0, -1):
            sT = st.tile([32, 32], F32, name=f"sT{u}")
            nc.vector.transpose(sT[:], scur[u][:])
            pc = psc.tile([32, 32], F32, space="PSUM", tag="pc")
            nc.tensor.matmul(pc[0:32, :], sT[0:32, :], cmats[u + 1][0:32, :],
                             start=True, stop=True)
            cm = st.tile([32, 32], F32, name=f"cm{u}")
            nc.vector.tensor_copy(cm[:], pc[:])
            cmats[u] = cm

        # G matrix: full suffix at t = W2 - 1: G = S^{u*}_{W2-1} . C_{u*+1}
        ustar = next(u for u in range(NSUB)
                     if SUB[u][0] <= W2 - 1 < SUB[u][1])
        sT31 = st.tile([32, 32], F32)
        nc.vector.transpose(sT31[:], scomp[:, 32 * (W2 - 1):32 * W2])
        pg = psc.tile([32, 32], F32, space="PSUM", tag="pg")
        nc.tensor.matmul(pg[0:32, :], sT31[0:32, :], cmats[ustar + 1][0:32, :],
                         start=True, stop=True)
        gsb = st.tile([32, 32], F32)
        nc.vector.tensor_copy(gsb[:], pg[:])
        nc.sync.dma_start(g_d[:, :], gsb[:])

        # tag columns: w_t = S^u_t^T iota, then per sub-chain join
        ptw = psc.tile([32, S2], F32, space="PSUM", tag="ptw")
        for t in range(S2):
            nc.tensor.matmul(
                ptw[0:32, t:t + 1], scomp[0:NT, 32 * t:32 * t + 32],
                iotacf[0:NT, 0:1], start=True, stop=True,
            )
        wtags = st.tile([32, S2], F32)
        nc.vector.tensor_copy(wtags[:], ptw[:])
        ptag = psc.tile([32, S2], F32, space="PSUM", tag="ptag")
        for u in range(NSUB):
            us = slice(SUB[u][0], SUB[u][1])
            nc.tensor.matmul(ptag[0:32, us], cmats[u + 1][0:NT, :],
                             wtags[0:NT, us], start=True, stop=True)
        tags = st.tile([32, S2], F32)
        nc.vector.tensor_copy(tags[:], ptag[:])
        nc.sync.dma_start(tags_d[:, :], tags[:])
    nc.compile()
    return nc


# --------------------------------------------------------------------------
# L3: cross-chunk stitch
# --------------------------------------------------------------------------
def build_l3():
    nc = _new_nc(1)
    NIN = 8 * S2 + 8 * 32 + 1
    in_d = nc.dram_tensor("stitchin", [32, NIN], F32, kind="ExternalInput").ap()
    path_d = nc.dram_tensor("path", [1, L], I32, kind="ExternalOutput").ap()

    with tile.TileContext(nc) as tc, ExitStack() as ctx:
        st = ctx.enter_context(tc.tile_pool(name="st", bufs=1))
        psum = ctx.enter_context(tc.tile_pool(name="psum", bufs=2, space="PSUM"))

        allin = st.tile([32, NIN], F32)
        nc.sync.dma_start(allin[:], in_d[:, :])
        tags = allin[:, 0:8 * S2]
        gt = allin[:, 8 * S2:8 * S2 + 8 * 32]
        term = allin[:, NIN - 1:NIN]

        # chunk-end one-hots: v[7] = term; v[c-1] = G_c . v[c] via lhsT=G_c^T
        vsb = st.tile([32, 8], F32)
        nc.vector.tensor_copy(vsb[:, 7:8], term)
        for c in range(7, 0, -1):
            pv = psum.tile([32, 1], F32, space="PSUM", tag="pv")
            nc.tensor.matmul(pv[0:32, :],
                             allin[0:NT, 8 * S2 + 32 * c:8 * S2 + 32 * c + 32],
                             vsb[0:NT, c:c + 1], start=True, stop=True)
            nc.vector.tensor_copy(vsb[:, c - 1:c], pv[:])

        pp = psum.tile([32, L], F32, space="PSUM", tag="pp")
        for c in range(8):
            base = c * S2 + (0 if c == 0 else W2)
            lo = 0 if c == 0 else S2 + (c - 1) * CL2
            ln = S2 if c == 0 else CL2
            nc.tensor.matmul(pp[0:1, lo:lo + ln], vsb[0:NT, c:c + 1],
                             allin[0:NT, base:base + ln], start=True, stop=True)
        path_sb = st.tile([1, L], I32)
        nc.vector.tensor_copy(path_sb[:], pp[0:1, :])
        nc.sync.dma_start(path_d[:, :], path_sb[:])
    nc.compile()
    return nc


# --------------------------------------------------------------------------
# host glue
# --------------------------------------------------------------------------
def _get(name, builder):
    if name not in _CACHE:
        _CACHE[name] = builder()
    return _CACHE[name]


def _prep_l1_core(sentence, embed_table, wih, bih, bhh, whh, h0, c0, k, reverse):
    """Inputs for one L1 core: direction weights + 8 chain windows."""
    import ml_dtypes
    s = np.asarray(sentence, np.int64)
    if reverse:
        s = s[::-1]
    # chain c on core k handles chunk q = k*NCH + c, window [CL1*q, CL1*q+S1)
    pos = np.zeros(NCH * SP1, np.int64)
    for c in range(NCH):
        q = k * NCH + c
        s0 = CL1 * q
        pos[c * SP1:c * SP1 + S1] = np.arange(s0, s0 + S1)
    inds = s[pos]
    ins = {
        "emb": np.asarray(embed_table, np.float32),
        "sent": np.ascontiguousarray(
            np.asarray(inds, np.int32).reshape(NG, 128).T),
    }
    w = np.asarray(wih, np.float32)[_PERM].copy()    # [2048, 300] g,i,f,o
    b = (np.asarray(bih, np.float32) + np.asarray(bhh, np.float32))[_PERM].copy()
    w[0:H] *= 2.0   # g-gate x2 for tanh(g) = 2*sigmoid(2g)-1
    b[0:H] *= 2.0
    wT = np.ascontiguousarray(w.T)                   # [300, 2048]
    ins["wA"] = np.ascontiguousarray(
        np.concatenate([wT[0:128], wT[128:256]], axis=1)).astype(ml_dtypes.bfloat16)
    ins["wB"] = np.ascontiguousarray(wT[256:300]).astype(ml_dtypes.bfloat16)
    ins["wC"] = np.ascontiguousarray(b[None, :]).astype(ml_dtypes.bfloat16)
    wh = np.asarray(whh, np.float32)[_PERM].copy()   # [2048, 512]
    wh[0:H] *= 2.0
    whT = np.ascontiguousarray(wh.T)                 # [512, 2048]
    ins["wpack"] = np.ascontiguousarray(
        whT.reshape(NK, 128, G4).transpose(1, 0, 2).reshape(128, NK * G4)
    ).astype(ml_dtypes.bfloat16)
    h0c = np.zeros((128, NK * NCH), np.float32)
    c0c = np.zeros((128, NK * NCH), np.float32)
    if k == 0:
        # chain 0 is chunk 0: true initial state; (j, c) layout
        h0j = np.asarray(h0, np.float32).reshape(NK, 128).T  # [128, NK]
        c0j = np.asarray(c0, np.float32).reshape(NK, 128).T
        for j in range(NK):
            h0c[:, j * NCH] = h0j[:, j]
            c0c[:, j * NCH] = c0j[:, j]
    ins["h0c"] = h0c.astype(ml_dtypes.bfloat16)
    ins["c0c"] = c0c
    return ins


def _assemble_h(results, reverse):
    """Assemble full h history [128, NK, L] (time order) for one direction."""
    hfull = np.zeros((128, NK, L), np.float32)
    for k in range(4):
        out = np.asarray(results[k], np.float32).reshape(128, S1, NK, NCH)
        for c in range(NCH):
            q = k * NCH + c
            wq = 0 if q == 0 else W1
            lo = CL1 * q + wq
            hi = CL1 * q + S1
            hfull[:, :, lo:hi] = np.moveaxis(out[:, wq:, :, c], 1, 2)
    if reverse:
        hfull = hfull[:, :, ::-1]
    return hfull


def _prep_l2_core(hf, hb, wout, bout, transitions, c2):
    import ml_dtypes
    w0 = CL2 * c2
    win = slice(w0, w0 + S2)
    hcat = np.zeros((128, 8 * SP2), np.float32)
    for j in range(NK):
        hcat[:, j * SP2:j * SP2 + S2] = hf[:, j, win]
        hcat[:, (NK + j) * SP2:(NK + j) * SP2 + S2] = hb[:, j, win]
    woT = np.ascontiguousarray(np.asarray(wout, np.float32).T)  # [1024, 20]
    wop = np.ascontiguousarray(
        np.concatenate([woT[j * 128:(j + 1) * 128] for j in range(8)], axis=1))
    trTp = np.zeros((32, 32), np.float32)
    trTp[0:NT, 0:NT] = np.asarray(transitions, np.float32).T
    fvi = np.zeros((32, 1), np.float32)
    if c2 == 0:
        fvi[0:NT, 0] = NEG
        fvi[START, 0] = 0.0
    return {
        "hcat": hcat.astype(ml_dtypes.bfloat16),
        "woutp": wop.astype(ml_dtypes.bfloat16),
        "bout": np.ascontiguousarray(
            np.asarray(bout, np.float32)[None, :]).astype(ml_dtypes.bfloat16),
        "transTp": trTp,
        "fvinit": fvi,
    }


def kernel(sentence, embed_table, w_ih_f, w_hh_f, b_ih_f, b_hh_f,
           w_ih_b, w_hh_b, b_ih_b, b_hh_b, h0, c0, w_out, b_out, transitions):
    h0 = np.asarray(h0, np.float32)
    c0 = np.asarray(c0, np.float32)

    # ---- L1: 8 cores, 32 LSTM chunks per direction
    nc1 = _get("l1", build_l1)
    ins1 = []
    for core in range(8):
        rev = core >= 4
        k = core % 4
        if rev:
            ins1.append(_prep_l1_core(sentence, embed_table, w_ih_b, b_ih_b,
                                      b_hh_b, w_hh_b, h0[1], c0[1], k, True))
        else:
            ins1.append(_prep_l1_core(sentence, embed_table, w_ih_f, b_ih_f,
                                      b_hh_f, w_hh_f, h0[0], c0[0], k, False))
    r1 = run_bass_kernel_spmd(nc1, ins1, core_ids=list(range(8))).results
    hf = _assemble_h([r1[k]["hT_out"] for k in range(4)], False)
    hb = _assemble_h([r1[4 + k]["hT_out"] for k in range(4)], True)

    # ---- L2: 8 cores, chunked CRF scan + composition
    nc2 = _get("l2", build_l2)
    ins2 = [_prep_l2_core(hf, hb, w_out, b_out, transitions, c2)
            for c2 in range(8)]
    r2 = run_bass_kernel_spmd(nc2, ins2, core_ids=list(range(8))).results

    # ---- L3: stitch
    nc3 = _get("l3", build_l3)
    tagsall = np.concatenate([np.asarray(r2[c]["tags"], np.float32)
                              for c in range(8)], axis=1)
    gtall = np.concatenate(
        [np.ascontiguousarray(np.asarray(r2[c]["gmat"], np.float32).T)
         for c in range(8)], axis=1)
    ins3 = {"stitchin": np.ascontiguousarray(np.concatenate(
        [tagsall, gtall, np.asarray(r2[7]["term"], np.float32)], axis=1))}
    r3 = run_bass_kernel_spmd(nc3, [ins3], core_ids=[0]).results[0]
    return np.ascontiguousarray(r3["path"].reshape(L)).astype(np.int32)


# revision 41
# speedup vs baseline: 1.0259x; 1.0251x over previous
"""BiLSTM-CRF Trainium2 kernel (Bass/Tile), three SPMD launches.

The 512-step LSTM recurrence and the 512-step CRF Viterbi scan are both
latency-chain bound on-chip (each step is a serial PE->ACT->DVE chain).
Both recurrences are exponentially forgetting, so they are chunked across
all 8 cores with warm-up prefixes that converge to the sequential
trajectory far below the (already path-exact) bf16 noise floor:

  L1 (8 cores): per direction, 32 LSTM chunks of 15 steps with a 32-step
      zero-state warm-up (chunk 0 starts from the true h0/c0 and is 47
      long). Each core runs its 8 chunks in lockstep: the chain index is
      a free-axis column, so one Ldweights+Matmult pair serves all 8
      chains ([128,8] moving operand) and the sigmoid/tanh/elementwise
      ops batch across chains. tanh(g) is computed as 2*sigmoid(2g)-1
      (g weights pre-scaled by 2 on the host, exact in bf16), so a step
      is: one sigmoid over i,f,g, one sigmoid over o, 4 DVE ops, one
      tanh, one h-multiply.
  L2 (8 cores): CRF Viterbi scan in 8 chunks of 62 steps with a 16-step
      zero-state warm-up (chunk 0 is 78 long, exact START init). Each
      core then composes its chunk's backpointer one-hot maps into
      suffix products on PE (6 concurrent sub-chains), emitting per-step
      "tag given chunk-end-tag" columns and the whole-chunk map G.
  L3 (1 core): chains the 8 chunk maps G to pick each chunk's end tag,
      then selects each chunk's tag columns with one matmul per chunk.

Host work is sharding glue: dtype casts, weight re-layout (incl. the x2
g-gate scaling), window slicing, time reversal for the backward
direction, and output concat.
"""

import numpy as np
from contextlib import ExitStack

import concourse.bass as bass
import concourse.tile as tile
from concourse import bacc, mybir
from concourse.bass_utils import run_bass_kernel_spmd
from concourse.masks import make_identity

F32 = mybir.dt.float32
I32 = mybir.dt.int32
U32 = mybir.dt.uint32
BF16 = mybir.dt.bfloat16
AF = mybir.ActivationFunctionType
OP = mybir.AluOpType

V, E, H, L = 100000, 300, 512, 512
NT, START, STOP, NEG = 20, 18, 19, -10000.0
G4 = 4 * H   # 2048
NM = G4 // 128  # 16 gate column-chunks
NK = H // 128   # 4 h row-chunks

# --- L1 chunking: 32 chunks per direction, 4 cores x 8 lockstep chains ---
NCH = 8       # chains per core (lockstep, chain = free-axis column)
W1 = 32       # LSTM warm-up steps
S1 = 47       # steps per chain (= W1 + CL1 = chunk-0 length)
CL1 = S1 - W1  # 15
SP1 = 48      # padded per-chain stride in the gathered window (8*48=384)
NG = 3        # gather tiles of 128 positions
NPOS = NG * 128  # 384
assert S1 + 31 * CL1 == L and NCH * SP1 == NPOS

# --- L2 chunking: 8 chunks, 1 per core ---
W2 = 16
S2 = 78       # scan steps per core (= W2 + CL2 = chunk-0 length)
CL2 = S2 - W2  # 62
SP2 = 80      # padded hcat window stride
NSUB = 6      # composition sub-chains
SUB = [(13 * u, 13 * (u + 1)) for u in range(NSUB)]
assert S2 + 7 * CL2 == L and SUB[-1][1] == S2

# gate row order used on-chip: g, i, f, o (sigmoid over g2 fires first)
_PERM = np.concatenate([
    np.arange(2 * H, 3 * H),  # g
    np.arange(0, H),          # i
    np.arange(H, 2 * H),      # f
    np.arange(3 * H, 4 * H),  # o
])

_CACHE: dict = {}


def _new_nc(num_devices):
    return bacc.Bacc(
        "TRN2", target_bir_lowering=False, debug=False, num_devices=num_devices
    )


# --------------------------------------------------------------------------
# L1: gather + input projection + 8 lockstep LSTM chunk recurrences
# --------------------------------------------------------------------------
def build_l1():
    nc = _new_nc(8)
    emb_d = nc.dram_tensor("emb", [V, E], F32, kind="ExternalInput").ap()
    sent_d = nc.dram_tensor("sent", [128, NG], I32, kind="ExternalInput").ap()
    wA_d = nc.dram_tensor("wA", [128, 2 * G4], BF16, kind="ExternalInput").ap()
    wB_d = nc.dram_tensor("wB", [E - 256, G4], BF16, kind="ExternalInput").ap()
    wC_d = nc.dram_tensor("wC", [1, G4], BF16, kind="ExternalInput").ap()
    wp_d = nc.dram_tensor("wpack", [128, NK * G4], BF16, kind="ExternalInput").ap()
    h0_d = nc.dram_tensor("h0c", [128, NK * NCH], BF16, kind="ExternalInput").ap()
    c0_d = nc.dram_tensor("c0c", [128, NK * NCH], F32, kind="ExternalInput").ap()
    hT_d = nc.dram_tensor("hT_out", [128, S1 * NK * NCH], BF16,
                          kind="ExternalOutput").ap()

    with tile.TileContext(nc) as tc, ExitStack() as ctx:
        const = ctx.enter_context(tc.tile_pool(name="const", bufs=1))
        state = ctx.enter_context(tc.tile_pool(name="state", bufs=1))
        ew = ctx.enter_context(tc.tile_pool(name="ew", bufs=4))

        ident = const.tile([128, 128], F32)
        make_identity(nc, ident[:])
        # x-projection, one tile per gate group so the recurrence can
        # start as soon as the g-group columns are written:
        # xpG (g: m 0..4), xpIF (i,f: m 4..12), xpB (o: m 12..16),
        # layout [128, (t*gm + (m-m0))*NCH + c]
        xpG = const.tile([128, SP1 * 4 * NCH], F32)
        xpIF = const.tile([128, SP1 * 8 * NCH], F32)
        xpB = const.tile([128, SP1 * 4 * NCH], F32)

        # --- phase A: gather + transpose + input projection ---
        # (index DMA first so the gathers don't queue behind the weights)
        phase_a = ExitStack()
        pxp = phase_a.enter_context(tc.tile_pool(name="pxp", bufs=3, space="PSUM"))
        ptp = phase_a.enter_context(tc.tile_pool(name="ptp", bufs=2, space="PSUM"))
        ones = const.tile([1, NPOS], BF16)
        nc.gpsimd.memset(ones[:], 1.0)
        idx = const.tile([128, NG], I32)
        nc.sync.dma_start(idx[:], sent_d[:, :])
        xg = []
        for g in range(NG):
            t = const.tile([128, E], F32, tag=f"xg{g}", name=f"xg{g}")
            nc.gpsimd.indirect_dma_start(
                out=t[:], out_offset=None, in_=emb_d[:, :],
                in_offset=bass.IndirectOffsetOnAxis(ap=idx[:, g:g + 1], axis=0),
            )
            xg.append(t)
        ecs = [128, 128, E - 256]
        xT = const.tile([128, 3 * NPOS], BF16)
        for e in range(3):
            e0 = sum(ecs[:e])
            for g in range(NG):
                pt = ptp.tile([128, 128], F32, space="PSUM", tag="pt")
                nc.tensor.transpose(
                    out=pt[0:ecs[e], :], in_=xg[g][:, e0:e0 + ecs[e]],
                    identity=ident[:],
                )
                nc.vector.tensor_copy(
                    xT[0:ecs[e], e * NPOS + g * 128: e * NPOS + (g + 1) * 128],
                    pt[0:ecs[e], :],
                )
        # weight loads go through the same (Pool/SWDGE) queue as the
        # gathers, AFTER them, so the gathers aren't stuck behind 3 MB of
        # weights on the DMA engines. rowi artificially depends on idx so
        # the scheduler cannot hoist the weight loads above the gathers.
        rowi0 = const.tile([128, 1], I32)
        nc.gpsimd.iota(rowi0[:], pattern=[[0, 1]], base=0, channel_multiplier=1)
        rowi = const.tile([128, 1], I32)
        nc.vector.scalar_tensor_tensor(
            out=rowi[:], in0=idx[:, 0:1], scalar=0, in1=rowi0[:],
            op0=OP.mult, op1=OP.add,
        )
        wa_sb = const.tile([128, 2 * G4], BF16)
        nc.gpsimd.indirect_dma_start(
            out=wa_sb[:], out_offset=None, in_=wA_d[:, :],
            in_offset=bass.IndirectOffsetOnAxis(ap=rowi[:, 0:1], axis=0))
        wb_sb = const.tile([E - 256, G4], BF16)
        nc.gpsimd.indirect_dma_start(
            out=wb_sb[:], out_offset=None, in_=wB_d[:, :],
            in_offset=bass.IndirectOffsetOnAxis(ap=rowi[0:E - 256, 0:1], axis=0))
        wc_sb = const.tile([1, G4], BF16)
        nc.sync.dma_start(wc_sb[:], wC_d[:, :])
        wp = const.tile([128, NK * G4], BF16)
        nc.gpsimd.indirect_dma_start(
            out=wp[:], out_offset=None, in_=wp_d[:, :],
            in_offset=bass.IndirectOffsetOnAxis(ap=rowi[:, 0:1], axis=0))
        xpvG = xpG[:].rearrange("p (t m c) -> p t m c", m=4, c=NCH)
        xpvIF = xpIF[:].rearrange("p (t m c) -> p t m c", m=8, c=NCH)
        xpvB = xpB[:].rearrange("p (t m c) -> p t m c", m=4, c=NCH)
        for m in range(NM):
            px = pxp.tile([128, NPOS], F32, space="PSUM", tag="px")
            ms = slice(m * 128, (m + 1) * 128)
            nc.tensor.matmul(px[:], wa_sb[:, ms], xT[0:128, 0:NPOS],
                             start=True, stop=False)
            nc.tensor.matmul(px[:], wa_sb[:, G4 + m * 128: G4 + (m + 1) * 128],
                             xT[0:128, NPOS:2 * NPOS], start=False, stop=False)
            nc.tensor.matmul(px[:], wb_sb[0:E - 256, ms],
                             xT[0:E - 256, 2 * NPOS:3 * NPOS],
                             start=False, stop=False)
            nc.tensor.matmul(px[:], wc_sb[0:1, ms], ones[0:1, :],
                             start=False, stop=True)
            # px columns are (c-major, t-minor); scatter to (t, m, c)
            pxv = px[:].rearrange("p (c t) -> p t c", c=NCH)
            if m < 4:
                dstv = xpvG[:, :, m, :]
            elif m < 12:
                dstv = xpvIF[:, :, m - 4, :]
            else:
                dstv = xpvB[:, :, m - 12, :]
            if m % 2 == 0:
                nc.vector.tensor_copy(dstv, pxv)
            else:
                nc.scalar.copy(dstv, pxv)
        phase_a.close()

        h0c = const.tile([128, NK * NCH], BF16)
        nc.sync.dma_start(h0c[:], h0_d[:, :])
        ones32 = const.tile([128, NK * NCH], F32)
        nc.gpsimd.memset(ones32[:], 1.0)

        psum = ctx.enter_context(tc.tile_pool(name="psum", bufs=2, space="PSUM"))
        psumg = ctx.enter_context(tc.tile_pool(name="psumg", bufs=3, space="PSUM"))

        c_sb = state.tile([128, NK * NCH], F32)   # (j, c) layout
        nc.sync.dma_start(c_sb[:], c0_d[:, :])
        hT = state.tile([128, S1 * NK * NCH], BF16)  # [(t*NK+j)*NCH+c]

        NB = NK * NCH  # 32

        def hblk(t, j):
            if t < 0:
                return h0c[:, j * NCH:(j + 1) * NCH]
            o = (t * NK + j) * NCH
            return hT[:, o:o + NCH]

        def gate_mms(pg, m0, m1, t, xpt):
            gm = m1 - m0
            nc.tensor.matmul(pg[:], ident[:],
                             xpt[:, t * gm * NCH:(t + 1) * gm * NCH],
                             start=True, stop=False)
            for m in range(m0, m1):
                for j in range(NK):
                    nc.tensor.matmul(
                        pg[:, (m - m0) * NCH:(m - m0 + 1) * NCH],
                        wp[:, j * G4 + m * 128: j * G4 + (m + 1) * 128],
                        hblk(t - 1, j), start=False,
                        stop=(m == m1 - 1 and j == NK - 1),
                    )

        for t in range(S1):
            # gate pre-activations: G = g2 (32), IF = i,f (64), B = o (32)
            pgG = psumg.tile([128, 4 * NCH], F32, space="PSUM", tag="pgG")
            pgIF = psum.tile([128, 8 * NCH], F32, space="PSUM", tag="pgIF")
            pgB = psum.tile([128, 4 * NCH], F32, space="PSUM", tag="pgB")
            gate_mms(pgG, 0, 4, t, xpG)
            gate_mms(pgIF, 4, 12, t, xpIF)
            gate_mms(pgB, 12, 16, t, xpB)
            gG = ew.tile([128, 4 * NCH], F32, tag="gG")
            nc.scalar.activation(gG[:], pgG[:], AF.Sigmoid)
            gIF = ew.tile([128, 8 * NCH], F32, tag="gIF")
            nc.scalar.activation(gIF[:], pgIF[:], AF.Sigmoid)
            gB = ew.tile([128, 4 * NCH], F32, tag="gB")
            nc.scalar.activation(gB[:], pgB[:], AF.Sigmoid)
            w4 = ew.tile([128, NB], F32, tag="w4")
            nc.vector.scalar_tensor_tensor(
                out=w4[:], in0=gG[:], scalar=2.0, in1=ones32[:],
                op0=OP.mult, op1=OP.subtract,
            )
            t2 = ew.tile([128, NB], F32, tag="t2")
            nc.vector.tensor_mul(t2[:], gIF[:, NB:2 * NB], c_sb[:])
            t1 = ew.tile([128, NB], F32, tag="t1")
            nc.vector.tensor_mul(t1[:], gIF[:, 0:NB], w4[:])
            nc.vector.tensor_add(c_sb[:], t1[:], t2[:])
            tcc = ew.tile([128, NB], F32, tag="tcc")
            nc.scalar.activation(tcc[:], c_sb[:], AF.Tanh)
            nc.vector.tensor_mul(hT[:, t * NB:(t + 1) * NB], gB[:], tcc[:])

        nc.sync.dma_start(hT_d[:, :], hT[:])
    nc.compile()
    return nc


# --------------------------------------------------------------------------
# L2: feats + chunked CRF scan + backpointer suffix composition
# --------------------------------------------------------------------------
def build_l2():
    nc = _new_nc(8)
    hcat_d = nc.dram_tensor("hcat", [128, 8 * SP2], BF16, kind="ExternalInput").ap()
    wo_d = nc.dram_tensor("woutp", [128, 8 * NT], BF16, kind="ExternalInput").ap()
    bo_d = nc.dram_tensor("bout", [1, NT], BF16, kind="ExternalInput").ap()
    tr_d = nc.dram_tensor("transTp", [32, 32], F32, kind="ExternalInput").ap()
    fv_d = nc.dram_tensor("fvinit", [32, 1], F32, kind="ExternalInput").ap()
    tags_d = nc.dram_tensor("tags", [32, S2], F32, kind="ExternalOutput").ap()
    g_d = nc.dram_tensor("gmat", [32, 32], F32, kind="ExternalOutput").ap()
    term_d = nc.dram_tensor("term", [32, 1], F32, kind="ExternalOutput").ap()

    with tile.TileContext(nc) as tc, ExitStack() as ctx:
        const = ctx.enter_context(tc.tile_pool(name="const", bufs=1))
        st = ctx.enter_context(tc.tile_pool(name="st", bufs=1))

        ident = const.tile([32, 32], F32)
        make_identity(nc, ident[:])
        hcat = const.tile([128, 8 * SP2], BF16)
        nc.sync.dma_start(hcat[:], hcat_d[:, :])
        wo = const.tile([128, 8 * NT], BF16)
        nc.sync.dma_start(wo[:], wo_d[:, :])
        bo = const.tile([1, NT], BF16)
        nc.sync.dma_start(bo[:], bo_d[:, :])
        trT = const.tile([32, 32], F32)
        nc.sync.dma_start(trT[:], tr_d[:, :])
        fvi = const.tile([32, 1], F32)
        nc.sync.dma_start(fvi[:], fv_d[:, :])
        ones = const.tile([1, SP2], BF16)
        nc.gpsimd.memset(ones[:], 1.0)

        # feats^T [20, SP2]
        phase_f = ExitStack()
        psf = phase_f.enter_context(tc.tile_pool(name="psf", bufs=1, space="PSUM"))
        pf = psf.tile([32, SP2], F32, space="PSUM", tag="pf")
        for j in range(8):
            nc.tensor.matmul(
                pf[0:NT, :], wo[:, j * NT:(j + 1) * NT],
                hcat[:, j * SP2:(j + 1) * SP2], start=(j == 0), stop=False,
            )
        nc.tensor.matmul(pf[0:NT, :], bo[0:1, :], ones[0:1, :],
                         start=False, stop=True)
        feats = st.tile([32, SP2], F32)
        nc.gpsimd.memset(feats[:], 0.0)
        nc.vector.tensor_copy(feats[0:NT, :], pf[0:NT, :])
        phase_f.close()

        # CRF forward scan over S2 steps, with the backpointer one-hot
        # builds and suffix-composition links of each sub-chain emitted as
        # soon as the scan has produced that sub-chain's backpointers (the
        # copies alternate DVE/ACT and fill the scan's engine slack).
        scT = st.tile([32, 32], F32)
        nc.gpsimd.memset(scT[:], 0.0)
        bpt = st.tile([32, 8 * S2], U32)
        schist = st.tile([32, 32 * S2], F32)
        mxhist = st.tile([32, 8 * S2], F32)
        nc.gpsimd.memset(mxhist[:], 0.0)
        iotar = st.tile([32, 32], I32)
        nc.gpsimd.iota(iotar[:], pattern=[[1, 32]], base=0, channel_multiplier=0)
        iotarf = st.tile([32, 32], F32)
        nc.vector.tensor_copy(iotarf[:], iotar[:])
        iotac = st.tile([32, 1], I32)
        nc.gpsimd.iota(iotac[:], pattern=[[0, 1]], base=0, channel_multiplier=1)
        iotacf = st.tile([32, 1], F32)
        nc.vector.tensor_copy(iotacf[:], iotac[:])
        bpf = st.tile([32, S2], F32)
        mall = st.tile([32, S2 * 32], BF16)
        scomp = st.tile([32, S2 * 32], BF16)
        identb = st.tile([32, 32], BF16)
        nc.vector.tensor_copy(identb[:], ident[:])
        iotacb = st.tile([32, 1], BF16)
        nc.vector.tensor_copy(iotacb[:], iotacf[:])
        sc = ctx.enter_context(tc.tile_pool(name="sc", bufs=3))
        phase_l = ExitStack()
        psl = phase_l.enter_context(tc.tile_pool(name="psl", bufs=1, space="PSUM"))
        scur = [None] * NSUB
        nlink = [0]

        nc.vector.tensor_scalar_add(scT[:, 0:NT], trT[:, 0:NT], fvi[:, 0:1])
        mx = None
        for t in range(S2):
            sct = schist[:, 32 * t:32 * (t + 1)]
            nc.vector.transpose(sct, scT[:])
            mx = mxhist[:, 8 * t:8 * t + 8]
            nc.vector.max(mx[0:NT, :], sct[0:NT, 0:NT])
            if t < S2 - 1:
                nc.vector.scalar_tensor_tensor(
                    out=scT[:, 0:NT], in0=trT[:, 0:NT], scalar=mx[:, 0:1],
                    in1=feats[:, t:t + 1].to_broadcast([32, NT]),
                    op0=OP.add, op1=OP.add,
                )
            nc.vector.max_index(
                bpt[0:NT, 8 * t:8 * t + 8], mxhist[0:NT, 8 * t:8 * t + 8],
                schist[0:NT, 32 * t:32 * t + NT],
            )

        # backpointer one-hot maps: mall[p, t*32+n] = (bptr_t[p] == n)
        nc.vector.tensor_copy(
            bpf[0:NT, :],
            bpt[0:NT, :].rearrange("p (t e) -> p t e", e=8)[:, :, 0],
        )
        nc.vector.tensor_tensor(
            out=mall[0:NT, :].rearrange("p (t n) -> p t n", n=32),
            in0=bpf[0:NT, :].rearrange("p (t o) -> p t o", o=1)
                .broadcast_to([NT, S2, 32]),
            in1=iotarf[0:NT, :].rearrange("p (o n) -> p o n", o=1)
                .broadcast_to([NT, S2, 32]),
            op=OP.is_equal,
        )
        for u in range(NSUB):
            nc.scalar.copy(
                scomp[:, 32 * (SUB[u][1] - 1):32 * SUB[u][1]], identb[:])

        # suffix-composition links, round-robin across sub-chains
        for step in range(SUB[0][1] - SUB[0][0]):
            for u in range(NSUB):
                u_lo, u_hi = SUB[u]
                t = u_hi - 1 - step
                srcs = scomp[:, 32 * t:32 * (t + 1)]
                ps = psl.tile([32, 32], F32, space="PSUM", tag=f"ps{u}")
                nc.tensor.matmul(
                    ps[0:32, :], mall[0:NT, t * 32:(t + 1) * 32],
                    srcs[0:NT, :], start=True, stop=True,
                )
                if t > u_lo:
                    dst = scomp[:, 32 * (t - 1):32 * t]
                else:
                    nxt = sc.tile([32, 32], BF16, tag=f"sc{u}", name=f"sloc{u}")
                    scur[u] = nxt
                    dst = nxt[:]
                if nlink[0] % 2 == 0:
                    nc.vector.tensor_copy(dst, ps[:])
                else:
                    nc.scalar.copy(dst, ps[:])
                nlink[0] += 1
        phase_l.close()

        # terminal one-hot (valid on core 7)
        phase_t = ExitStack()
        pst = phase_t.enter_context(tc.tile_pool(name="pst", bufs=1, space="PSUM"))
        term = st.tile([32, 1], F32)
        nc.gpsimd.memset(term[:], NEG)
        nc.vector.scalar_tensor_tensor(
            out=term[0:NT, :], in0=trT[0:NT, STOP:STOP + 1],
            scalar=mx[0:NT, 0:1], in1=feats[0:NT, S2 - 1:S2],
            op0=OP.add, op1=OP.add,
        )
        t32 = st.tile([32, 32], F32)
        nc.gpsimd.memset(t32[:], NEG)
        nc.vector.tensor_copy(t32[:, 0:1], term[:])
        tT = st.tile([32, 32], F32)
        nc.vector.transpose(tT[:], t32[:])
        mxt = st.tile([32, 8], F32)
        nc.vector.max(mxt[0:1, :], tT[0:1, 0:NT])
        onesf = st.tile([1, NT], F32)
        nc.gpsimd.memset(onesf[:], 1.0)
        pmx = pst.tile([32, 1], F32, space="PSUM", tag="pmx")
        nc.tensor.matmul(pmx[0:NT, :], onesf[0:1, 0:NT], mxt[0:1, 0:1],
                         start=True, stop=True)
        mxb = st.tile([32, 1], F32)
        nc.vector.tensor_copy(mxb[0:NT, :], pmx[0:NT, :])
        termOH = st.tile([32, 1], F32)
        nc.gpsimd.memset(termOH[:], 0.0)
        nc.vector.tensor_scalar(
            termOH[0:NT, :], term[0:NT, :], mxb[0:NT, 0:1], None, OP.is_equal,
        )
        nc.sync.dma_start(term_d[:, :], termOH[:])
        phase_t.close()

        # cross-chain products C_u = S^u_loc . C_{u+1}, C_NSUB = identity
        psc = ctx.enter_context(tc.tile_pool(name="psc", bufs=1, space="PSUM"))
        cmats = [None] * (NSUB + 1)
        cmats[NSUB] = identb
        for u in range(NSUB - 1, 0, -1):
            sT = st.tile([32, 32], BF16, name=f"sT{u}")
            nc.vector.transpose(sT[:], scur[u][:])
            pc = psc.tile([32, 32], F32, space="PSUM", tag="pc")
            nc.tensor.matmul(pc[0:32, :], sT[0:32, :], cmats[u + 1][0:32, :],
                             start=True, stop=True)
            cm = st.tile([32, 32], BF16, name=f"cm{u}")
            nc.vector.tensor_copy(cm[:], pc[:])
            cmats[u] = cm

        # G matrix: full suffix at t = W2 - 1: G = S^{u*}_{W2-1} . C_{u*+1}
        ustar = next(u for u in range(NSUB)
                     if SUB[u][0] <= W2 - 1 < SUB[u][1])
        sT31 = st.tile([32, 32], BF16)
        nc.vector.transpose(sT31[:], scomp[:, 32 * (W2 - 1):32 * W2])
        pg = psc.tile([32, 32], F32, space="PSUM", tag="pg")
        nc.tensor.matmul(pg[0:32, :], sT31[0:32, :], cmats[ustar + 1][0:32, :],
                         start=True, stop=True)
        gsb = st.tile([32, 32], F32)
        nc.vector.tensor_copy(gsb[:], pg[:])
        nc.sync.dma_start(g_d[:, :], gsb[:])

        # tag columns: w_t = S^u_t^T iota, then per sub-chain join
        ptw = psc.tile([32, S2], F32, space="PSUM", tag="ptw")
        for t in range(S2):
            nc.tensor.matmul(
                ptw[0:32, t:t + 1], scomp[0:NT, 32 * t:32 * t + 32],
                iotacb[0:NT, 0:1], start=True, stop=True,
            )
        wtags = st.tile([32, S2], BF16)
        nc.vector.tensor_copy(wtags[:], ptw[:])
        ptag = psc.tile([32, S2], F32, space="PSUM", tag="ptag")
        for u in range(NSUB):
            us = slice(SUB[u][0], SUB[u][1])
            nc.tensor.matmul(ptag[0:32, us], cmats[u + 1][0:NT, :],
                             wtags[0:NT, us], start=True, stop=True)
        tags = st.tile([32, S2], F32)
        nc.vector.tensor_copy(tags[:], ptag[:])
        nc.sync.dma_start(tags_d[:, :], tags[:])
    nc.compile()
    return nc


# --------------------------------------------------------------------------
# L3: cross-chunk stitch
# --------------------------------------------------------------------------
def build_l3():
    nc = _new_nc(1)
    NIN = 8 * S2 + 8 * 32 + 1
    in_d = nc.dram_tensor("stitchin", [32, NIN], BF16, kind="ExternalInput").ap()
    path_d = nc.dram_tensor("path", [1, L], I32, kind="ExternalOutput").ap()

    with tile.TileContext(nc) as tc, ExitStack() as ctx:
        st = ctx.enter_context(tc.tile_pool(name="st", bufs=1))
        psum = ctx.enter_context(tc.tile_pool(name="psum", bufs=2, space="PSUM"))
        psumg = ctx.enter_context(tc.tile_pool(name="psumg", bufs=3, space="PSUM"))

        allin = st.tile([32, NIN], BF16)
        nc.sync.dma_start(allin[:], in_d[:, :])
        tags = allin[:, 0:8 * S2]
        gt = allin[:, 8 * S2:8 * S2 + 8 * 32]
        term = allin[:, NIN - 1:NIN]

        # chunk-end one-hots: v[7] = term; v[c-1] = G_c . v[c] via lhsT=G_c^T
        vsb = st.tile([32, 8], BF16)
        nc.vector.tensor_copy(vsb[:, 7:8], term)
        for c in range(7, 0, -1):
            pv = psum.tile([32, 1], F32, space="PSUM", tag="pv")
            nc.tensor.matmul(pv[0:32, :],
                             allin[0:NT, 8 * S2 + 32 * c:8 * S2 + 32 * c + 32],
                             vsb[0:NT, c:c + 1], start=True, stop=True)
            nc.vector.tensor_copy(vsb[:, c - 1:c], pv[:])

        pp = psum.tile([32, L], F32, space="PSUM", tag="pp")
        for c in range(8):
            base = c * S2 + (0 if c == 0 else W2)
            lo = 0 if c == 0 else S2 + (c - 1) * CL2
            ln = S2 if c == 0 else CL2
            nc.tensor.matmul(pp[0:1, lo:lo + ln], vsb[0:NT, c:c + 1],
                             allin[0:NT, base:base + ln], start=True, stop=True)
        path_sb = st.tile([1, L], I32)
        nc.vector.tensor_copy(path_sb[:], pp[0:1, :])
        nc.sync.dma_start(path_d[:, :], path_sb[:])
    nc.compile()
    return nc


# --------------------------------------------------------------------------
# host glue
# --------------------------------------------------------------------------
def _get(name, builder):
    if name not in _CACHE:
        _CACHE[name] = builder()
    return _CACHE[name]


def _prep_l1_core(sentence, embed_table, wih, bih, bhh, whh, h0, c0, k, reverse):
    """Inputs for one L1 core: direction weights + 8 chain windows."""
    import ml_dtypes
    s = np.asarray(sentence, np.int64)
    if reverse:
        s = s[::-1]
    # chain c on core k handles chunk q = k*NCH + c, window [CL1*q, CL1*q+S1)
    pos = np.zeros(NCH * SP1, np.int64)
    for c in range(NCH):
        q = k * NCH + c
        s0 = CL1 * q
        pos[c * SP1:c * SP1 + S1] = np.arange(s0, s0 + S1)
    inds = s[pos]
    ins = {
        "emb": np.asarray(embed_table, np.float32),
        "sent": np.ascontiguousarray(
            np.asarray(inds, np.int32).reshape(NG, 128).T),
    }
    w = np.asarray(wih, np.float32)[_PERM].copy()    # [2048, 300] g,i,f,o
    b = (np.asarray(bih, np.float32) + np.asarray(bhh, np.float32))[_PERM].copy()
    w[0:H] *= 2.0   # g-gate x2 for tanh(g) = 2*sigmoid(2g)-1
    b[0:H] *= 2.0
    wT = np.ascontiguousarray(w.T)                   # [300, 2048]
    ins["wA"] = np.ascontiguousarray(
        np.concatenate([wT[0:128], wT[128:256]], axis=1)).astype(ml_dtypes.bfloat16)
    ins["wB"] = np.ascontiguousarray(wT[256:300]).astype(ml_dtypes.bfloat16)
    ins["wC"] = np.ascontiguousarray(b[None, :]).astype(ml_dtypes.bfloat16)
    wh = np.asarray(whh, np.float32)[_PERM].copy()   # [2048, 512]
    wh[0:H] *= 2.0
    whT = np.ascontiguousarray(wh.T)                 # [512, 2048]
    ins["wpack"] = np.ascontiguousarray(
        whT.reshape(NK, 128, G4).transpose(1, 0, 2).reshape(128, NK * G4)
    ).astype(ml_dtypes.bfloat16)
    h0c = np.zeros((128, NK * NCH), np.float32)
    c0c = np.zeros((128, NK * NCH), np.float32)
    if k == 0:
        # chain 0 is chunk 0: true initial state; (j, c) layout
        h0j = np.asarray(h0, np.float32).reshape(NK, 128).T  # [128, NK]
        c0j = np.asarray(c0, np.float32).reshape(NK, 128).T
        for j in range(NK):
            h0c[:, j * NCH] = h0j[:, j]
            c0c[:, j * NCH] = c0j[:, j]
    ins["h0c"] = h0c.astype(ml_dtypes.bfloat16)
    ins["c0c"] = c0c
    return ins


def _assemble_h(results, reverse):
    """Assemble full h history [128, NK, L] (time order) for one direction."""
    hfull = np.zeros((128, NK, L), np.float32)
    for k in range(4):
        out = np.asarray(results[k], np.float32).reshape(128, S1, NK, NCH)
        for c in range(NCH):
            q = k * NCH + c
            wq = 0 if q == 0 else W1
            lo = CL1 * q + wq
            hi = CL1 * q + S1
            hfull[:, :, lo:hi] = np.moveaxis(out[:, wq:, :, c], 1, 2)
    if reverse:
        hfull = hfull[:, :, ::-1]
    return hfull


def _prep_l2_core(hf, hb, wout, bout, transitions, c2):
    import ml_dtypes
    w0 = CL2 * c2
    win = slice(w0, w0 + S2)
    hcat = np.zeros((128, 8 * SP2), np.float32)
    for j in range(NK):
        hcat[:, j * SP2:j * SP2 + S2] = hf[:, j, win]
        hcat[:, (NK + j) * SP2:(NK + j) * SP2 + S2] = hb[:, j, win]
    woT = np.ascontiguousarray(np.asarray(wout, np.float32).T)  # [1024, 20]
    wop = np.ascontiguousarray(
        np.concatenate([woT[j * 128:(j + 1) * 128] for j in range(8)], axis=1))
    trTp = np.zeros((32, 32), np.float32)
    trTp[0:NT, 0:NT] = np.asarray(transitions, np.float32).T
    fvi = np.zeros((32, 1), np.float32)
    if c2 == 0:
        fvi[0:NT, 0] = NEG
        fvi[START, 0] = 0.0
    return {
        "hcat": hcat.astype(ml_dtypes.bfloat16),
        "woutp": wop.astype(ml_dtypes.bfloat16),
        "bout": np.ascontiguousarray(
            np.asarray(bout, np.float32)[None, :]).astype(ml_dtypes.bfloat16),
        "transTp": trTp,
        "fvinit": fvi,
    }


def kernel(sentence, embed_table, w_ih_f, w_hh_f, b_ih_f, b_hh_f,
           w_ih_b, w_hh_b, b_ih_b, b_hh_b, h0, c0, w_out, b_out, transitions):
    h0 = np.asarray(h0, np.float32)
    c0 = np.asarray(c0, np.float32)

    # ---- L1: 8 cores, 32 LSTM chunks per direction
    nc1 = _get("l1", build_l1)
    ins1 = []
    for core in range(8):
        rev = core >= 4
        k = core % 4
        if rev:
            ins1.append(_prep_l1_core(sentence, embed_table, w_ih_b, b_ih_b,
                                      b_hh_b, w_hh_b, h0[1], c0[1], k, True))
        else:
            ins1.append(_prep_l1_core(sentence, embed_table, w_ih_f, b_ih_f,
                                      b_hh_f, w_hh_f, h0[0], c0[0], k, False))
    r1 = run_bass_kernel_spmd(nc1, ins1, core_ids=list(range(8))).results
    hf = _assemble_h([r1[k]["hT_out"] for k in range(4)], False)
    hb = _assemble_h([r1[4 + k]["hT_out"] for k in range(4)], True)

    # ---- L2: 8 cores, chunked CRF scan + composition
    nc2 = _get("l2", build_l2)
    ins2 = [_prep_l2_core(hf, hb, w_out, b_out, transitions, c2)
            for c2 in range(8)]
    r2 = run_bass_kernel_spmd(nc2, ins2, core_ids=list(range(8))).results

    # ---- L3: stitch
    nc3 = _get("l3", build_l3)
    tagsall = np.concatenate([np.asarray(r2[c]["tags"], np.float32)
                              for c in range(8)], axis=1)
    gtall = np.concatenate(
        [np.ascontiguousarray(np.asarray(r2[c]["gmat"], np.float32).T)
         for c in range(8)], axis=1)
    import ml_dtypes
    ins3 = {"stitchin": np.ascontiguousarray(np.concatenate(
        [tagsall, gtall, np.asarray(r2[7]["term"], np.float32)],
        axis=1)).astype(ml_dtypes.bfloat16)}
    r3 = run_bass_kernel_spmd(nc3, [ins3], core_ids=[0]).results[0]
    return np.ascontiguousarray(r3["path"].reshape(L)).astype(np.int32)
